# revision 1
# baseline (speedup 1.0000x reference)
# kernel.py — DiscriminativeLoss on 8 TRN2 NeuronCores (Bass/Tile, SPMD).
#
# Math (matches reference):
#   counts_k = #{i: l_i = k};  S_k = sum_{i in k} x_i;  mu_k = S_k / max(c_k, 1)
#   intra = (1/K) * sum_i invc_{l_i} * relu(||x_i - (mu - eps)|| - 1.5)^2
#   inter = sum_{a != b} relu(1 - ||(mu_a + eps) - mu_b||)^2 / (K*(K-1))
#   reg   = (1/K) * sum_k ||mu_k + eps||
#   total = intra + inter + 0.001 * reg
#
# Device strategy (per core, data-parallel over points):
#   - points-on-partitions layout: point i = p*TPC + j lives at [p, j]
#   - pass 1: one-hot H [128,64] per tile via DVE is_equal; PE matmul
#     lhsT=[X|1] [128,33] x rhs=H [128,64] accumulates [33,64] = [S^T; counts]
#   - AllReduce [33,64] across 8 cores
#   - stats: transpose -> [64,33]; mu, invc, inter/reg losses (tiny ops)
#   - pass 2: transposed one-hot HT [64,128] per tile (built from a host-
#     provided tile-major label copy via DMA broadcast + is_equal) used as
#     matmul WEIGHTS against Table [64,33]=[mu-eps | invc] -> per-point
#     gathered rows [128,33] (mu_{l_i}-eps, invc_{l_i})
#   - diff = x - (mu-eps); dist2 = rowsum(diff^2); hinge; dot with invc
#   - AllReduce intra partial; assemble scalar
import math
import numpy as np
from contextlib import ExitStack

import concourse.bass as bass
import concourse.bacc as bacc
import concourse.tile as tile
import concourse.mybir as mybir
from concourse.bass_utils import run_bass_kernel_spmd

F32 = mybir.dt.float32
BF16 = mybir.dt.bfloat16
I16 = mybir.dt.int16

N_CORES = 8
K = 64
D = 32
P = 128
EPS = 1e-8
PAD_LABEL = 999  # never matches any one-hot column

INTRA_MARGIN = 1.5
INTER_MARGIN2 = 1.0  # 2 * 0.5


def _host_prep(features, labels, tpc):
    """Shard + relayout on host. Returns per-core input dicts."""
    n_total = features.shape[0]
    n_core = n_total // N_CORES
    n_pad = P * tpc
    import ml_dtypes

    in_maps = []
    for c in range(N_CORES):
        f = np.asarray(features[c * n_core : (c + 1) * n_core], dtype=np.float32)
        l = np.asarray(labels[c * n_core : (c + 1) * n_core], dtype=np.int64)
        if n_pad > n_core:
            f = np.concatenate([f, np.zeros((n_pad - n_core, D), np.float32)], axis=0)
            l = np.concatenate([l, np.full((n_pad - n_core,), PAD_LABEL, np.int64)])
        # Xe: [P, tpc, 33] bf16, col 32 = 1.0
        xe = np.ones((n_pad, D + 1), np.float32)
        xe[:, :D] = f
        xe = xe.reshape(P, tpc, D + 1).astype(ml_dtypes.bfloat16)
        # p-major labels (for pass-1 one-hot): [P, tpc] int16, NO offset
        l_pm = l.reshape(P, tpc).astype(np.int16)
        # tile-major labels (for pass-2 transposed one-hot), paired:
        # A-set = tiles [0, na), B-set = tiles [na, tpc) with +64 offset.
        na = (tpc + 1) // 2
        ltm = l.reshape(P, tpc).T.astype(np.int16)  # [tpc, P]
        l_tma = np.ascontiguousarray(ltm[:na]).ravel()
        ltmb = np.full((na, P), PAD_LABEL, np.int16)
        ltmb[: tpc - na] = ltm[na:] + 64
        l_tmb = np.ascontiguousarray(ltmb).ravel()
        in_maps.append(
            {
                "xe": np.ascontiguousarray(xe),
                "labels_pm": np.ascontiguousarray(l_pm),
                "labels_tma": l_tma,
                "labels_tmb": l_tmb,
                "iota64": np.tile(np.arange(K, dtype=np.int16), (P, 1)),
                "labels_pmf": l_pm.astype(np.float32),
                "iota64f": np.tile(np.arange(K, dtype=np.float32), (P, 1)),
                "iotacol": np.arange(P, dtype=np.float32).reshape(P, 1),
                "id33": np.eye(D + 1, dtype=np.float32),
                "id64": np.eye(K, dtype=np.float32),
                "eyeneg": (1.0 - np.eye(K, dtype=np.float32)).astype(
                    ml_dtypes.bfloat16
                ),
            }
        )
    return in_maps


def build_program(tpc, j1=20, j2=8, stage=3):
    """Build the SPMD Bass program. tpc = tiles per core (cols per partition)."""
    nc = bacc.Bacc(
        "TRN2", target_bir_lowering=False, debug=False, num_devices=N_CORES
    )
    core_ids = list(range(N_CORES))

    xe_d = nc.dram_tensor("xe", [P, tpc, D + 1], BF16, kind="ExternalInput").ap()
    lpm_d = nc.dram_tensor("labels_pm", [P, tpc], I16, kind="ExternalInput").ap()
    lpmf_d = nc.dram_tensor("labels_pmf", [P, tpc], F32, kind="ExternalInput").ap()
    iota64f_d = nc.dram_tensor("iota64f", [P, K], F32, kind="ExternalInput").ap()
    na = (tpc + 1) // 2
    ltma_d = nc.dram_tensor("labels_tma", [na * P], I16, kind="ExternalInput").ap()
    ltmb_d = nc.dram_tensor("labels_tmb", [na * P], I16, kind="ExternalInput").ap()
    iota64_d = nc.dram_tensor("iota64", [P, K], I16, kind="ExternalInput").ap()
    iotacol_d = nc.dram_tensor("iotacol", [P, 1], F32, kind="ExternalInput").ap()
    id33_d = nc.dram_tensor("id33", [D + 1, D + 1], F32, kind="ExternalInput").ap()
    id64_d = nc.dram_tensor("id64", [K, K], F32, kind="ExternalInput").ap()
    eyeneg_d = nc.dram_tensor("eyeneg", [K, K], BF16, kind="ExternalInput").ap()
    out_d = nc.dram_tensor("out", [3], F32, kind="ExternalOutput").ap()

    with tile.TileContext(nc, num_cores=N_CORES) as tc, ExitStack() as ctx:
        singles = ctx.enter_context(tc.tile_pool(name="singles", bufs=1))
        xpool = ctx.enter_context(tc.tile_pool(name="xpool", bufs=1))
        hpool = ctx.enter_context(tc.tile_pool(name="hpool", bufs=4))
        htpool = ctx.enter_context(tc.tile_pool(name="htpool", bufs=4))
        l2pool = ctx.enter_context(tc.tile_pool(name="l2pool", bufs=4))
        wpool = ctx.enter_context(tc.tile_pool(name="wpool", bufs=3))
        psA = ctx.enter_context(tc.tile_pool(name="psA", bufs=1, space="PSUM"))
        psMg = ctx.enter_context(tc.tile_pool(name="psMg", bufs=4, space="PSUM"))
        psS = ctx.enter_context(tc.tile_pool(name="psS", bufs=3, space="PSUM"))
        dram = ctx.enter_context(tc.tile_pool(name="dram", bufs=2, space="DRAM"))

        # ---------- constants ----------
        iota64 = singles.tile([P, K], I16)
        nc.sync.dma_start(out=iota64, in_=iota64_d)
        id33 = singles.tile([D + 1, D + 1], F32)
        nc.sync.dma_start(out=id33, in_=id33_d)
        id64 = singles.tile([K, K], F32)
        nc.sync.dma_start(out=id64, in_=id64_d)
        eyeneg = singles.tile([K, K], BF16)
        nc.sync.dma_start(out=eyeneg, in_=eyeneg_d)
        iotacol = singles.tile([P, 1], F32)  # = partition index p (0..127)
        nc.sync.dma_start(out=iotacol, in_=iotacol_d)
        epsneg = singles.tile([P, 1], F32)
        nc.vector.memset(epsneg, -EPS)
        epspos = singles.tile([P, 1], F32)
        nc.vector.memset(epspos, EPS)
        margneg = singles.tile([P, 1], F32)
        nc.vector.memset(margneg, -float(INTRA_MARGIN))
        xe = xpool.tile([P, tpc, D + 1], BF16)
        lpm = singles.tile([P, tpc], I16)
        nc.sync.dma_start(out=lpm, in_=lpm_d)
        lpmf = singles.tile([P, tpc], F32)
        nc.sync.dma_start(out=lpmf, in_=lpmf_d)
        iota64f = singles.tile([P, K], F32)
        nc.sync.dma_start(out=iota64f, in_=iota64f_d)

        # ---------- pass 1: segment sums ----------
        psumS = psA.tile([D + 1, K], F32)
        n_chunks1 = math.ceil(tpc / j1)
        t_done = 0
        for c in range(n_chunks1):
            j0 = c * j1
            jn = min(j1, tpc - j0)
            # stream X chunk
            xq = nc.sync if (c % 2 == 0) else nc.scalar
            xq.dma_start(
                out=xe[:, j0 : j0 + jn, :], in_=xe_d[:, j0 : j0 + jn, :]
            )
            h = hpool.tile([P, j1, K], BF16, tag="h")
            if c % 4 == 3:
                half = (jn + 1) // 2
                for s0 in range(0, jn, half):
                    sn = min(half, jn - s0)
                    g0 = j0 + s0
                    tdf = hpool.tile([P, half, K], F32, tag="gtmp")
                    nc.gpsimd.tensor_sub(
                        tdf[:, :sn, :],
                        lpmf[:, g0 : g0 + sn, None].to_broadcast((P, sn, K)),
                        iota64f[:, None, :].to_broadcast((P, sn, K)),
                    )
                    usq = hpool.tile([P, half, K], F32, tag="gtmp")
                    nc.gpsimd.tensor_mul(
                        usq[:, :sn, :], tdf[:, :sn, :], tdf[:, :sn, :]
                    )
                    nc.gpsimd.tensor_scalar_min(
                        usq[:, :sn, :], usq[:, :sn, :], 1.0
                    )
                    nc.gpsimd.tensor_scalar(
                        h[:, s0 : s0 + sn, :], usq[:, :sn, :], -1.0, 1.0,
                        mybir.AluOpType.mult, mybir.AluOpType.add,
                    )
            else:
                nc.vector.tensor_tensor(
                    h[:, :jn, :],
                    lpm[:, j0 : j0 + jn, None].to_broadcast((P, jn, K)),
                    iota64[:, None, :].to_broadcast((P, jn, K)),
                    mybir.AluOpType.is_equal,
                )
            for j in range(jn):
                nc.tensor.matmul(
                    psumS,
                    xe[:, j0 + j, :],
                    h[:, j, :],
                    start=(t_done == 0),
                    stop=(t_done == tpc - 1),
                )
                t_done += 1

        # ---------- AllReduce segment sums ----------
        sg_local = wpool.tile([D + 1, K], F32, tag="sg")
        nc.scalar.copy(out=sg_local, in_=psumS)
        cc_in = dram.tile([D + 1, K], F32)
        cc_out = dram.tile([D + 1, K], F32)
        nc.gpsimd.dma_start(out=cc_in, in_=sg_local)
        nc.gpsimd.collective_compute(
            "AllReduce",
            mybir.AluOpType.add,
            replica_groups=[core_ids],
            ins=[cc_in.opt()],
            outs=[cc_out.opt()],
        )
        sg = wpool.tile([D + 1, K], F32, tag="sg2")
        nc.gpsimd.dma_start(out=sg, in_=cc_out)
        if stage == 1:
            nc.sync.dma_start(out=out_d, in_=sg[0:1, 0:1])

        # ---------- stats: mu, invc, Table, inter, reg ----------
        run_stats = stage >= 2
        # transpose [33, 64] -> [64, 33]
        psW = psS.tile([K, D + 1], F32, tag="small")
        nc.tensor.transpose(psW, sg, id33)
        W = wpool.tile([K, D + 1], F32, tag="w")  # [S_k | c_k]
        nc.scalar.copy(out=W, in_=psW)
        safec = wpool.tile([K, 1], F32, tag="safec")
        nc.vector.tensor_scalar_max(safec, W[:, D : D + 1], 1.0)
        invc = wpool.tile([K, 1], F32, tag="invc")
        nc.vector.reciprocal(invc, safec)
        mu = wpool.tile([K, D], F32, tag="mu")
        nc.vector.tensor_mul(mu, W[:, :D], invc.to_broadcast((K, D)))
        mum = wpool.tile([K, D], F32, tag="mum")  # mu - eps
        nc.vector.tensor_scalar_add(mum, mu, -EPS)
        mup = wpool.tile([K, D], F32, tag="mup")  # mu + eps
        nc.vector.tensor_scalar_add(mup, mu, EPS)
        # q = ||mu||^2, qp = ||mu+eps||^2  (per cluster)
        qsc = wpool.tile([K, D], F32, tag="qsc")
        nc.vector.tensor_mul(qsc, mu, mu)
        q = wpool.tile([K, 1], F32, tag="q")
        nc.vector.tensor_reduce(
            out=q, in_=qsc, axis=mybir.AxisListType.X, op=mybir.AluOpType.add
        )
        qpsc = wpool.tile([K, D], F32, tag="qpsc")
        nc.vector.tensor_mul(qpsc, mup, mup)
        qp = wpool.tile([K, 1], F32, tag="qp")
        nc.vector.tensor_reduce(
            out=qp, in_=qpsc, axis=mybir.AxisListType.X, op=mybir.AluOpType.add
        )
        # Table [64, 33] bf16 = [mu - eps | invc]
        table = singles.tile([P, D + 1], BF16)
        nc.scalar.copy(out=table[:K, :D], in_=mum)
        nc.scalar.copy(out=table[:K, D : D + 1], in_=invc)
        # replicate rows [0,64) -> [64,128) for the B-side matmuls
        nc.sync.dma_start(out=table[K:, :], in_=table[:K, :])

        # inter: pd2[a,b] = qp_a - 2*mup_a.mu_b + q_b
        ab = wpool.tile([K, D + 2], F32, tag="ab")  # [-2*mup | qp | 1]
        nc.scalar.mul(out=ab[:, :D], in_=mup, mul=-2.0)
        nc.scalar.copy(out=ab[:, D : D + 1], in_=qp)
        nc.vector.memset(ab[:, D + 1 : D + 2], 1.0)
        bb = wpool.tile([K, D + 2], F32, tag="bb")  # [mu | 1 | q]
        nc.scalar.copy(out=bb[:, :D], in_=mu)
        nc.vector.memset(bb[:, D : D + 1], 1.0)
        nc.scalar.copy(out=bb[:, D + 1 : D + 2], in_=q)
        psT = psS.tile([D + 2, K], F32, tag="small")
        nc.tensor.transpose(psT, ab, id64)
        atp = wpool.tile([D + 2, K], F32, tag="atp")
        nc.scalar.copy(out=atp, in_=psT)
        psT2 = psS.tile([D + 2, K], F32, tag="small")
        nc.tensor.transpose(psT2, bb, id64)
        btp = wpool.tile([D + 2, K], F32, tag="btp")
        nc.scalar.copy(out=btp, in_=psT2)
        psPD = psS.tile([K, K], F32, tag="small")
        nc.tensor.matmul(psPD, atp, btp)
        pdc = wpool.tile([K, K], F32, tag="pdc")
        nc.vector.tensor_scalar_max(pdc, psPD, 0.0)
        pdist = wpool.tile([K, K], F32, tag="pdist")
        nc.scalar.activation(
            out=pdist, in_=pdc, func=mybir.ActivationFunctionType.Sqrt
        )
        hingeI = wpool.tile([K, K], F32, tag="hingeI")
        nc.scalar.activation(
            out=hingeI, in_=pdist, func=mybir.ActivationFunctionType.Relu,
            bias=float(INTER_MARGIN2), scale=-1.0,
        )
        hm = wpool.tile([K, K], F32, tag="hm")
        nc.vector.tensor_mul(hm, hingeI, eyeneg)
        hm2 = wpool.tile([K, K], F32, tag="hm2")
        nc.vector.tensor_mul(hm2, hm, hm)
        interp = wpool.tile([K, 1], F32, tag="interp")
        nc.vector.tensor_reduce(
            out=interp, in_=hm2, axis=mybir.AxisListType.X, op=mybir.AluOpType.add
        )
        # reg rows: sqrt(qp)
        sqp = wpool.tile([K, 1], F32, tag="sqp")
        nc.scalar.activation(
            out=sqp, in_=qp, func=mybir.ActivationFunctionType.Sqrt
        )
        # partition sums of [interp | sqp] via matmul with ones
        cat2 = wpool.tile([K, 2], F32, tag="cat2")
        nc.scalar.copy(out=cat2[:, 0:1], in_=interp)
        nc.scalar.copy(out=cat2[:, 1:2], in_=sqp)
        ones64 = singles.tile([K, 1], F32)
        nc.vector.memset(ones64, 1.0)
        psIR = psS.tile([1, 2], F32, tag="small")
        nc.tensor.matmul(psIR, ones64, cat2)
        ir = wpool.tile([1, 2], F32, tag="ir")  # [inter_sum, reg_sum]
        nc.scalar.copy(out=ir, in_=psIR)
        if stage == 2:
            nc.sync.dma_start(out=out_d, in_=ir[0:1, 0:1])

        # ---------- pass 2: per-point gather + hinge ----------
        d2all = singles.tile([P, tpc], F32)
        invc_all = singles.tile([P, tpc], F32)
        TPAIR = 30       # pairs per outer chunk (l2/ht granularity)
        JMG = 15         # pairs per PSUM sub-chunk
        nb = tpc - na
        n_outer = math.ceil(na / TPAIR)
        for oc in range(n_outer):
            t0 = oc * TPAIR
            tn = min(TPAIR, na - t0)
            l2 = l2pool.tile([P, TPAIR * P], I16, tag="l2")
            dmaq = nc.sync if (oc % 2 == 0) else nc.scalar
            srcA = ltma_d[t0 * P : (t0 + tn) * P]
            dmaq.dma_start(
                out=l2[:K, : tn * P],
                in_=bass.AP(
                    tensor=srcA.tensor, offset=srcA.offset,
                    ap=[[0, K]] + [[int(s), int(n)] for s, n in srcA.ap],
                ),
            )
            srcB = ltmb_d[t0 * P : (t0 + tn) * P]
            dmaq.dma_start(
                out=l2[K:, : tn * P],
                in_=bass.AP(
                    tensor=srcB.tensor, offset=srcB.offset,
                    ap=[[0, K]] + [[int(s), int(n)] for s, n in srcB.ap],
                ),
            )
            ht = htpool.tile([P, TPAIR * P], BF16, tag="ht")
            nc.vector.tensor_single_scalar(
                ht[:, : tn * P], l2[:, : tn * P], iotacol,
                mybir.AluOpType.is_equal,
            )
            for ic in range(math.ceil(tn / JMG)):
                i0 = ic * JMG
                inn = min(JMG, tn - i0)
                a0 = t0 + i0                      # first A-tile index
                bn = max(0, min(inn, nb - a0))    # B-tiles that exist
                psmgA = psMg.tile([P, JMG, D + 1], F32, tag="psmg")
                for i in range(inn):
                    nc.tensor.matmul(
                        psmgA[:, i, :],
                        ht[:K, (i0 + i) * P : (i0 + i + 1) * P],
                        table[:K, :],
                    )
                dfA = hpool.tile([P, JMG, D], BF16, tag="df")
                nc.vector.tensor_sub(
                    dfA[:, :inn, :],
                    xe[:, a0 : a0 + inn, :D],
                    psmgA[:, :inn, :D],
                )
                nc.scalar.copy(
                    out=invc_all[:, a0 : a0 + inn], in_=psmgA[:, :inn, D]
                )
                sqA = hpool.tile([P, JMG, D], BF16, tag="sq")
                nc.scalar.activation(
                    out=sqA[:, :inn, :], in_=dfA[:, :inn, :],
                    func=mybir.ActivationFunctionType.Square,
                )
                nc.vector.tensor_reduce(
                    out=d2all[:, a0 : a0 + inn], in_=sqA[:, :inn, :],
                    axis=mybir.AxisListType.X, op=mybir.AluOpType.add,
                )
                if bn > 0:
                    b0 = na + a0                  # first B-tile index
                    psmgB = psMg.tile([P, JMG, D + 1], F32, tag="psmg")
                    for i in range(bn):
                        nc.tensor.matmul(
                            psmgB[:, i, :],
                            ht[K:, (i0 + i) * P : (i0 + i + 1) * P],
                            table[K:, :],
                        )
                    dfB = hpool.tile([P, JMG, D], BF16, tag="df")
                    nc.vector.tensor_sub(
                        dfB[:, :bn, :],
                        xe[:, b0 : b0 + bn, :D],
                        psmgB[:, :bn, :D],
                    )
                    nc.scalar.copy(
                        out=invc_all[:, b0 : b0 + bn], in_=psmgB[:, :bn, D]
                    )
                    sqB = hpool.tile([P, JMG, D], BF16, tag="sq")
                    nc.scalar.activation(
                        out=sqB[:, :bn, :], in_=dfB[:, :bn, :],
                        func=mybir.ActivationFunctionType.Square,
                    )
                    nc.vector.tensor_reduce(
                        out=d2all[:, b0 : b0 + bn], in_=sqB[:, :bn, :],
                        axis=mybir.AxisListType.X, op=mybir.AluOpType.add,
                    )

        # ---------- finals ----------
        dist = singles.tile([P, tpc], F32)
        nc.scalar.activation(
            out=dist, in_=d2all, func=mybir.ActivationFunctionType.Sqrt
        )
        nc.scalar.activation(
            out=dist, in_=dist, func=mybir.ActivationFunctionType.Relu,
            bias=margneg,
        )
        nc.vector.tensor_mul(d2all, dist, dist)
        nc.vector.tensor_mul(d2all, d2all, invc_all)
        rowsum = singles.tile([P, 1], F32)
        nc.vector.tensor_reduce(
            out=rowsum, in_=d2all, axis=mybir.AxisListType.X,
            op=mybir.AluOpType.add,
        )
        ones128 = singles.tile([P, 1], F32)
        nc.vector.memset(ones128, 1.0)
        psL = psS.tile([1, 1], F32, tag="small")
        nc.tensor.matmul(psL, rowsum, ones128)
        tot = wpool.tile([1, 3], F32, tag="tot")
        nc.scalar.copy(out=tot[:, 0:1], in_=psL)
        nc.scalar.copy(out=tot[:, 1:3], in_=ir)
        nc.sync.dma_start(out=out_d, in_=tot[0:1, :])

    nc.compile()
    return nc


_NC_CACHE = {}


def _get_program(tpc):
    if tpc not in _NC_CACHE:
        _NC_CACHE[tpc] = build_program(tpc)
    return _NC_CACHE[tpc]


def kernel(features, labels, num_clusters):
    features = np.asarray(features)
    labels = np.asarray(labels)
    n_total = features.shape[0]
    n_core = n_total // N_CORES
    tpc = math.ceil(n_core / P)
    nc = _get_program(tpc)
    in_maps = _host_prep(features, labels, tpc)
    res = run_bass_kernel_spmd(nc, in_maps, list(range(N_CORES)))
    intra_sum = sum(float(res.results[c]["out"][0]) for c in range(N_CORES))
    inter_sum = float(res.results[0]["out"][1])
    reg_sum = float(res.results[0]["out"][2])
    total = (
        intra_sum / K
        + inter_sum / (K * (K - 1))
        + 0.001 * reg_sum / K
    )
    return np.float32(total)



# revision 7
# speedup vs baseline: 1.5930x; 1.5930x over previous
# kernel.py — DiscriminativeLoss on 8 TRN2 NeuronCores (Bass/Tile, SPMD).
#
# Math (matches reference):
#   counts_k = #{i: l_i = k};  S_k = sum_{i in k} x_i;  mu_k = S_k / max(c_k, 1)
#   intra = (1/K) * sum_i invc_{l_i} * relu(||x_i - mu_{l_i} + eps|| - 1.5)^2
#   inter = sum_{a != b} relu(1 - ||(mu_a + eps) - mu_b||)^2 / (K*(K-1))
#   reg   = (1/K) * sum_k ||mu_k + eps||
#   total = intra + inter + 0.001 * reg
#
# Device strategy (per core, data-parallel over points; point i = p*tpc + j
# lives at [p, j]):
#   pass 1: one-hot H2 [128, 64, J1] built per chunk via a single DVE
#     tensor_tensor is_equal against a materialized replicated iota (all
#     operands packed 2-byte -> 2x DVE mode); per-tile PE matmul
#     lhsT=H2[:, :, j] [128, 64] x rhs=xe[:, j, :] [128, 33] accumulates
#     S^T|counts [64, 33] directly (N=33 -> cheap).
#   AllReduce [64, 33] across 8 cores (28us fixed cost; overlapped with
#     pass-2 one-hot prebuilds).
#   stats: invc = 1/max(c,1), mu = S*invc, table [128, 33] = [mu-eps | invc]
#     replicated to rows 64:128 for the B-half pairing.
#   pass 2, per outer chunk of 15 A-tiles + 15 B-tiles: transposed one-hot
#     ht [128, 15*128] built at 4x DVE (TensorScalarPtr is_equal vs the
#     per-partition iota) from a broadcast-DMA'd label row; 3 matmuls per
#     tile accumulate  psum[:, slot, 0:32] = gather(mu-eps) - x  (the diff
#     computed entirely on PE via a -Identity matmul) and
#     psum[:, slot, 32] = gather(invc); Act squares the PSUM diff to bf16;
#     DVE reduces via log2 halving adds (packed bf16 -> 2x mode).
#   finals: dist = sqrt(d2); hinge = relu(dist-1.5); intra partial =
#     sum hinge^2 * invc via 2 muls + row reduce + partition reduce.
#   inter/reg (KxK) replicated on every core from the reduced stats.
import math
import numpy as np
from contextlib import ExitStack

import concourse.bass as bass
import concourse.bacc as bacc
import concourse.tile as tile
import concourse.mybir as mybir
from concourse.bass_utils import run_bass_kernel_spmd

F32 = mybir.dt.float32
BF16 = mybir.dt.bfloat16
I16 = mybir.dt.int16

N_CORES = 8
K = 64
D = 32
P = 128
EPS = 1e-8
PAD_LABEL = 999  # never matches any one-hot row (0..127)

INTRA_MARGIN = 1.5
INTER_MARGIN2 = 1.0  # 2 * 0.5

J1 = 60       # pass-1 tiles per one-hot chunk
JMG = 15      # pass-2 A-tiles (and B-tiles) per outer chunk
POOL_EVERY = 4  # every 4th pass-1 chunk is built on the Pool engine
PREBUILD = 12   # pass-2 ht chunks emitted before the collective section
L2_BUFS = 8
HT_BUFS = 12


def _host_prep(features, labels, tpc):
    """Shard + relayout on host. Returns per-core input dicts."""
    n_total = features.shape[0]
    n_core = n_total // N_CORES
    n_pad = P * tpc
    import ml_dtypes

    na = (tpc + 1) // 2
    nout = math.ceil(na / JMG)
    iota_rep = np.tile(
        np.arange(K, dtype=np.int16)[None, :, None], (P, 1, J1)
    )
    iotacol = np.arange(P, dtype=np.float32).reshape(P, 1)
    negid = (-np.eye(P)).astype(ml_dtypes.bfloat16)
    id64 = np.eye(K, dtype=np.float32)
    eyeneg = (1.0 - np.eye(K, dtype=np.float32)).astype(ml_dtypes.bfloat16)

    in_maps = []
    for c in range(N_CORES):
        f = np.asarray(features[c * n_core : (c + 1) * n_core], dtype=np.float32)
        l = np.asarray(labels[c * n_core : (c + 1) * n_core], dtype=np.int64)
        if n_pad > n_core:
            f = np.concatenate([f, np.zeros((n_pad - n_core, D), np.float32)], axis=0)
            l = np.concatenate([l, np.full((n_pad - n_core,), PAD_LABEL, np.int64)])
        # xe: [P, tpc, 33] bf16, col 32 = 1.0
        xe = np.ones((n_pad, D + 1), np.float32)
        xe[:, :D] = f
        xe = xe.reshape(P, tpc, D + 1).astype(ml_dtypes.bfloat16)
        # p-major labels (pass-1 one-hot): [P, tpc] int16
        lpm = l.reshape(P, tpc).astype(np.int16)
        # tile-major labels for pass 2: ltm [nout, 2, JMG*P] int16,
        # [oc, 0] = A-tile labels, [oc, 1] = B-tile labels + 64.
        ltm_full = l.reshape(P, tpc).T.astype(np.int16)  # [tpc, P]
        ltm = np.full((nout, 2, JMG * P), PAD_LABEL, np.int16)
        for oc in range(nout):
            a0 = oc * JMG
            an = min(JMG, na - a0)
            ltm[oc, 0, : an * P] = ltm_full[a0 : a0 + an].ravel()
            b0 = na + a0
            bn = max(0, min(JMG, tpc - b0))
            if bn > 0:
                ltm[oc, 1, : bn * P] = ltm_full[b0 : b0 + bn].ravel() + 64
        in_maps.append(
            {
                "xe": np.ascontiguousarray(xe),
                "lpm": np.ascontiguousarray(lpm),
                "ltm": np.ascontiguousarray(ltm),
                "iota_rep": iota_rep,
                "iotacol": iotacol,
                "negid": negid,
                "id64": id64,
                "eyeneg": eyeneg,
            }
        )
    return in_maps


def build_program(tpc):
    """Build the SPMD Bass program. tpc = tiles per core (cols per partition)."""
    nc = bacc.Bacc(
        "TRN2", target_bir_lowering=False, debug=False, num_devices=N_CORES
    )
    core_ids = list(range(N_CORES))

    na = (tpc + 1) // 2
    nout = math.ceil(na / JMG)
    n_chunks1 = math.ceil(tpc / J1)

    xe_d = nc.dram_tensor("xe", [P, tpc, D + 1], BF16, kind="ExternalInput").ap()
    lpm_d = nc.dram_tensor("lpm", [P, tpc], I16, kind="ExternalInput").ap()
    ltm_d = nc.dram_tensor("ltm", [nout, 2, JMG * P], I16, kind="ExternalInput").ap()
    iota_rep_d = nc.dram_tensor("iota_rep", [P, K, J1], I16, kind="ExternalInput").ap()
    iotacol_d = nc.dram_tensor("iotacol", [P, 1], F32, kind="ExternalInput").ap()
    negid_d = nc.dram_tensor("negid", [P, P], BF16, kind="ExternalInput").ap()
    id64_d = nc.dram_tensor("id64", [K, K], F32, kind="ExternalInput").ap()
    eyeneg_d = nc.dram_tensor("eyeneg", [K, K], BF16, kind="ExternalInput").ap()
    out_d = nc.dram_tensor("out", [3], F32, kind="ExternalOutput").ap()

    with tile.TileContext(nc, num_cores=N_CORES) as tc, ExitStack() as ctx:
        singles = ctx.enter_context(tc.tile_pool(name="singles", bufs=1))
        xpool = ctx.enter_context(tc.tile_pool(name="xpool", bufs=1))
        hpool = ctx.enter_context(tc.tile_pool(name="hpool", bufs=2))
        l2pool = ctx.enter_context(tc.tile_pool(name="l2pool", bufs=L2_BUFS))
        htpool = ctx.enter_context(tc.tile_pool(name="htpool", bufs=HT_BUFS))
        sqpool = ctx.enter_context(tc.tile_pool(name="sqpool", bufs=2))
        hvpool = ctx.enter_context(tc.tile_pool(name="hvpool", bufs=2))
        wpool = ctx.enter_context(tc.tile_pool(name="wpool", bufs=2))
        psA = ctx.enter_context(tc.tile_pool(name="psA", bufs=1, space="PSUM"))
        psMg = ctx.enter_context(tc.tile_pool(name="psMg", bufs=3, space="PSUM"))
        psS = ctx.enter_context(tc.tile_pool(name="psS", bufs=1, space="PSUM"))
        dram = ctx.enter_context(tc.tile_pool(name="dram", bufs=2, space="DRAM"))

        # ---------- constants ----------
        lpm = singles.tile([P, tpc], I16)
        nc.sync.dma_start(out=lpm, in_=lpm_d)
        iota_rep = singles.tile([P, K, J1], I16)
        nc.sync.dma_start(out=iota_rep, in_=iota_rep_d)
        iotacol = singles.tile([P, 1], F32)
        nc.sync.dma_start(out=iotacol, in_=iotacol_d)
        negid = singles.tile([P, P], BF16)
        nc.sync.dma_start(out=negid, in_=negid_d)
        id64 = singles.tile([K, K], F32)
        nc.sync.dma_start(out=id64, in_=id64_d)
        eyeneg = singles.tile([K, K], BF16)
        nc.sync.dma_start(out=eyeneg, in_=eyeneg_d)
        margneg = singles.tile([P, 1], F32)
        nc.vector.memset(margneg, -float(INTRA_MARGIN))
        ones64 = singles.tile([K, 1], F32)
        nc.vector.memset(ones64, 1.0)
        xe = xpool.tile([P, tpc, D + 1], BF16)

        d2all = singles.tile([P, tpc], F32)
        invc_all = singles.tile([P, tpc], BF16)

        # ---------- pass 1: segment sums (S^T | counts) [64, 33] ----------
        psumS = psA.tile([K, D + 1], F32)
        t_done = 0
        for c in range(n_chunks1):
            j0 = c * J1
            jn = min(J1, tpc - j0)
            nc.sync.dma_start(
                out=xe[:, j0 : j0 + jn, :], in_=xe_d[:, j0 : j0 + jn, :]
            )
            h2 = hpool.tile([P, K, J1], BF16, tag="h2")
            eng = nc.gpsimd if (c % POOL_EVERY == POOL_EVERY - 1) else nc.vector
            eng.tensor_tensor(
                h2[:, :, :jn],
                lpm[:, None, j0 : j0 + jn].to_broadcast((P, K, jn)),
                iota_rep[:, :, :jn],
                mybir.AluOpType.is_equal,
            )
            for j in range(jn):
                nc.tensor.matmul(
                    psumS,
                    h2[:, :, j],
                    xe[:, j0 + j, :],
                    start=(t_done == 0),
                    stop=(t_done == tpc - 1),
                )
                t_done += 1

        # ---------- pass-2 prep: prebuild label rows + transposed one-hots ----
        # (no dependency on the collective -> fills the AllReduce window)
        l2_tiles = {}
        ht_tiles = {}

        def emit_l2_ht(oc):
            src = ltm_d[oc]
            l2 = l2pool.tile([P, JMG * P], I16, tag="l2")
            nc.sync.dma_start(
                out=l2,
                in_=bass.AP(
                    tensor=src.tensor,
                    offset=src.offset,
                    ap=[[JMG * P, 2], [0, K]] + [[1, JMG * P]],
                ),
            )
            ht = htpool.tile([P, JMG * P], BF16, tag="ht")
            nc.vector.tensor_single_scalar(
                ht, l2, iotacol, mybir.AluOpType.is_equal
            )
            l2_tiles[oc] = l2
            ht_tiles[oc] = ht

        for oc in range(min(PREBUILD, nout)):
            emit_l2_ht(oc)

        # ---------- AllReduce the [64, 33] stats ----------
        sg_local = wpool.tile([K, D + 1], F32, tag="sg")
        nc.scalar.copy(out=sg_local, in_=psumS)
        cc_in = dram.tile([K, D + 1], F32)
        cc_out = dram.tile([K, D + 1], F32)
        nc.gpsimd.dma_start(out=cc_in, in_=sg_local)
        nc.gpsimd.collective_compute(
            "AllReduce",
            mybir.AluOpType.add,
            replica_groups=[core_ids],
            ins=[cc_in.opt()],
            outs=[cc_out.opt()],
        )
        sg = singles.tile([K, D + 1], F32)
        nc.gpsimd.dma_start(out=sg, in_=cc_out)

        # ---------- critical stats: table = [mu - eps | invc] ----------
        safec = singles.tile([K, 1], F32)
        nc.vector.tensor_scalar_max(safec, sg[:, D : D + 1], 1.0)
        invc = singles.tile([K, 1], F32)
        nc.vector.reciprocal(invc, safec)
        mu = singles.tile([K, D], F32)
        nc.vector.tensor_mul(mu, sg[:, :D], invc.to_broadcast((K, D)))
        table = singles.tile([P, D + 1], BF16)
        nc.vector.tensor_scalar_add(table[:K, :D], mu, -EPS)
        nc.vector.tensor_scalar_add(table[:K, D : D + 1], invc, 0.0)
        # replicate rows [0,64) -> [64,128) for the B-half matmuls
        # (issued from the Pool queue, which is idle post-collective; keeping
        # it off SP avoids stalling the pass-2 label-row DMA stream)
        nc.gpsimd.dma_start(out=table[K:, :], in_=table[:K, :])

        # ---------- pass 2: diff via PE, square on Act, reduce on DVE ----------
        for oc in range(nout):
            a0 = oc * JMG
            an = min(JMG, na - a0)
            b0 = na + a0
            bn = max(0, min(JMG, tpc - b0))
            if oc not in ht_tiles:
                emit_l2_ht(oc)
            ht = ht_tiles.pop(oc)
            l2_tiles.pop(oc)

            # psmg [P, 32, 32] = exactly 2 psum banks; 32-wide diff slots never
            # cross a bank boundary. Slots 0:15 = A diffs, 15:30 = B diffs,
            # slot 30 col i = A-tile-i invc, slot 31 col i = B-tile-i invc.
            psmg = psMg.tile([P, 2 * JMG + 2, D], F32, tag="psmg")
            for i in range(an):
                hta = ht[:K, i * P : (i + 1) * P]
                # gather(mu-eps) opens the group; negid accumulates -x
                nc.tensor.matmul(
                    psmg[:, i, :], hta, table[:K, :D], start=True, stop=False
                )
                nc.tensor.matmul(
                    psmg[:, i, :], negid, xe[:, a0 + i, :D],
                    start=False, stop=True,
                )
                nc.tensor.matmul(
                    psmg[:, 2 * JMG, i : i + 1], hta, table[:K, D : D + 1],
                    start=True, stop=True,
                )
            for i in range(bn):
                htb = ht[K:, i * P : (i + 1) * P]
                s = JMG + i
                nc.tensor.matmul(
                    psmg[:, s, :], htb, table[K:, :D], start=True, stop=False
                )
                nc.tensor.matmul(
                    psmg[:, s, :], negid, xe[:, b0 + i, :D],
                    start=False, stop=True,
                )
                nc.tensor.matmul(
                    psmg[:, 2 * JMG + 1, i : i + 1], htb, table[K:, D : D + 1],
                    start=True, stop=True,
                )

            # invc gather extraction (Pool engine)
            nc.gpsimd.tensor_scalar_add(
                invc_all[:, a0 : a0 + an], psmg[:, 2 * JMG, :an], 0.0
            )
            if bn > 0:
                nc.gpsimd.tensor_scalar_add(
                    invc_all[:, b0 : b0 + bn], psmg[:, 2 * JMG + 1, :bn], 0.0
                )
            # square + halving-add reduction over D=32 (packed bf16 -> 2x DVE)
            sq = sqpool.tile([P, 2 * JMG, D], BF16, tag="sq")
            h16 = hvpool.tile([P, 2 * JMG, 16], BF16, tag="h16")
            h8 = hvpool.tile([P, 2 * JMG, 8], BF16, tag="h8")
            h4 = hvpool.tile([P, 2 * JMG, 4], BF16, tag="h4")
            h2t = hvpool.tile([P, 2 * JMG, 2], BF16, tag="h2t")
            if an == JMG and bn == JMG:
                ranges = [(0, 2 * JMG)]  # one combined pass over all 30 slots
            else:
                ranges = [(0, an)] + ([(JMG, bn)] if bn > 0 else [])
            for s0, n in ranges:
                nc.scalar.activation(
                    out=sq[:, s0 : s0 + n, :], in_=psmg[:, s0 : s0 + n, :],
                    func=mybir.ActivationFunctionType.Square,
                )
                nc.vector.tensor_add(
                    h16[:, s0 : s0 + n, :],
                    sq[:, s0 : s0 + n, 0:16], sq[:, s0 : s0 + n, 16:32],
                )
                nc.vector.tensor_add(
                    h8[:, s0 : s0 + n, :],
                    h16[:, s0 : s0 + n, 0:8], h16[:, s0 : s0 + n, 8:16],
                )
                nc.vector.tensor_add(
                    h4[:, s0 : s0 + n, :],
                    h8[:, s0 : s0 + n, 0:4], h8[:, s0 : s0 + n, 4:8],
                )
                nc.vector.tensor_add(
                    h2t[:, s0 : s0 + n, :],
                    h4[:, s0 : s0 + n, 0:2], h4[:, s0 : s0 + n, 2:4],
                )
            nc.vector.tensor_add(
                d2all[:, a0 : a0 + an], h2t[:, :an, 0], h2t[:, :an, 1]
            )
            if bn > 0:
                nc.vector.tensor_add(
                    d2all[:, b0 : b0 + bn],
                    h2t[:, JMG : JMG + bn, 0],
                    h2t[:, JMG : JMG + bn, 1],
                )

        # ---------- inter + reg (tiny, replicated) ----------
        mup = wpool.tile([K, D], F32, tag="mup")
        nc.vector.tensor_scalar_add(mup, mu, EPS)
        qsc = wpool.tile([K, D], F32, tag="qsc")
        nc.vector.tensor_mul(qsc, mu, mu)
        q = wpool.tile([K, 1], F32, tag="q")
        nc.vector.tensor_reduce(
            out=q, in_=qsc, axis=mybir.AxisListType.X, op=mybir.AluOpType.add
        )
        qpsc = wpool.tile([K, D], F32, tag="qpsc")
        nc.vector.tensor_mul(qpsc, mup, mup)
        qp = wpool.tile([K, 1], F32, tag="qp")
        nc.vector.tensor_reduce(
            out=qp, in_=qpsc, axis=mybir.AxisListType.X, op=mybir.AluOpType.add
        )
        # pd2[a,b] = qp_a - 2*mup_a.mu_b + q_b via one [64,64] matmul
        ab = wpool.tile([K, D + 2], F32, tag="ab")  # [-2*mup | qp | 1]
        nc.scalar.mul(out=ab[:, :D], in_=mup, mul=-2.0)
        nc.scalar.copy(out=ab[:, D : D + 1], in_=qp)
        nc.vector.memset(ab[:, D + 1 : D + 2], 1.0)
        bb = wpool.tile([K, D + 2], F32, tag="bb")  # [mu | 1 | q]
        nc.scalar.copy(out=bb[:, :D], in_=mu)
        nc.vector.memset(bb[:, D : D + 1], 1.0)
        nc.scalar.copy(out=bb[:, D + 1 : D + 2], in_=q)
        psT = psS.tile([D + 2, K], F32, tag="small")
        nc.tensor.transpose(psT, ab, id64)
        atp = wpool.tile([D + 2, K], F32, tag="atp")
        nc.scalar.copy(out=atp, in_=psT)
        psT2 = psS.tile([D + 2, K], F32, tag="small")
        nc.tensor.transpose(psT2, bb, id64)
        btp = wpool.tile([D + 2, K], F32, tag="btp")
        nc.scalar.copy(out=btp, in_=psT2)
        psPD = psS.tile([K, K], F32, tag="small")
        nc.tensor.matmul(psPD, atp, btp)
        pdc = wpool.tile([K, K], F32, tag="pdc")
        nc.vector.tensor_scalar_max(pdc, psPD, 0.0)
        pdist = wpool.tile([K, K], F32, tag="pdist")
        nc.scalar.activation(
            out=pdist, in_=pdc, func=mybir.ActivationFunctionType.Sqrt
        )
        hingeI = wpool.tile([K, K], F32, tag="hingeI")
        nc.scalar.activation(
            out=hingeI, in_=pdist, func=mybir.ActivationFunctionType.Relu,
            bias=float(INTER_MARGIN2), scale=-1.0,
        )
        hm = wpool.tile([K, K], F32, tag="hm")
        nc.vector.tensor_mul(hm, hingeI, eyeneg)
        hm2 = wpool.tile([K, K], F32, tag="hm2")
        nc.vector.tensor_mul(hm2, hm, hm)
        interp = wpool.tile([K, 1], F32, tag="interp")
        nc.vector.tensor_reduce(
            out=interp, in_=hm2, axis=mybir.AxisListType.X, op=mybir.AluOpType.add
        )
        sqp = wpool.tile([K, 1], F32, tag="sqp")
        nc.scalar.activation(
            out=sqp, in_=qp, func=mybir.ActivationFunctionType.Sqrt
        )
        cat2 = wpool.tile([K, 2], F32, tag="cat2")
        nc.scalar.copy(out=cat2[:, 0:1], in_=interp)
        nc.scalar.copy(out=cat2[:, 1:2], in_=sqp)
        psIR = psS.tile([1, 2], F32, tag="small")
        nc.tensor.matmul(psIR, ones64, cat2)
        ir = wpool.tile([1, 2], F32, tag="ir")  # [inter_sum, reg_sum]
        nc.scalar.copy(out=ir, in_=psIR)

        # ---------- intra finals ----------
        dist = singles.tile([P, tpc], BF16)
        nc.scalar.activation(
            out=dist, in_=d2all, func=mybir.ActivationFunctionType.Sqrt
        )
        hinge = singles.tile([P, tpc], BF16)
        nc.scalar.activation(
            out=hinge, in_=dist, func=mybir.ActivationFunctionType.Relu,
            bias=margneg,
        )
        hsq = singles.tile([P, tpc], BF16)
        nc.vector.tensor_mul(hsq, hinge, hinge)
        hv = singles.tile([P, tpc], F32)
        nc.vector.tensor_mul(hv, hsq, invc_all)
        rowsum = singles.tile([P, 1], F32)
        nc.vector.tensor_reduce(
            out=rowsum, in_=hv, axis=mybir.AxisListType.X,
            op=mybir.AluOpType.add,
        )
        intra = wpool.tile([1, 1], F32, tag="intra")
        nc.gpsimd.tensor_reduce(
            out=intra, in_=rowsum, axis=mybir.AxisListType.C,
            op=mybir.AluOpType.add,
        )
        tot = wpool.tile([1, 3], F32, tag="tot")
        nc.scalar.copy(out=tot[:, 0:1], in_=intra)
        nc.scalar.copy(out=tot[:, 1:3], in_=ir)
        nc.sync.dma_start(out=out_d, in_=tot[0:1, :])

    nc.compile()
    return nc


_NC_CACHE = {}


def _get_program(tpc):
    if tpc not in _NC_CACHE:
        _NC_CACHE[tpc] = build_program(tpc)
    return _NC_CACHE[tpc]


def kernel(features, labels, num_clusters):
    features = np.asarray(features)
    labels = np.asarray(labels)
    n_total = features.shape[0]
    n_core = n_total // N_CORES
    tpc = math.ceil(n_core / P)
    nc = _get_program(tpc)
    in_maps = _host_prep(features, labels, tpc)
    res = run_bass_kernel_spmd(nc, in_maps, list(range(N_CORES)))
    intra_sum = sum(float(res.results[c]["out"][0]) for c in range(N_CORES))
    inter_sum = float(res.results[0]["out"][1])
    reg_sum = float(res.results[0]["out"][2])
    total = (
        intra_sum / K
        + inter_sum / (K * (K - 1))
        + 0.001 * reg_sum / K
    )
    return np.float32(total)


# revision 16
# speedup vs baseline: 1.6248x; 1.0200x over previous
# kernel.py — DiscriminativeLoss on 8 TRN2 NeuronCores (Bass/Tile, SPMD).
#
# Math (matches reference):
#   counts_k = #{i: l_i = k};  S_k = sum_{i in k} x_i;  mu_k = S_k / max(c_k, 1)
#   intra = (1/K) * sum_i invc_{l_i} * relu(||x_i - mu_{l_i} + eps|| - 1.5)^2
#   inter = sum_{a != b} relu(1 - ||(mu_a + eps) - mu_b||)^2 / (K*(K-1))
#   reg   = (1/K) * sum_k ||mu_k + eps||
#   total = intra + inter + 0.001 * reg
#
# Device strategy (per core, data-parallel over points; point i = p*tpc + j
# lives at [p, j]):
#   pass 1: one-hot H2 [128, 64, J1] built per chunk via a single DVE
#     tensor_tensor is_equal against a materialized replicated iota (all
#     operands packed 2-byte -> 2x DVE mode); per-tile PE matmul
#     lhsT=H2[:, :, j] [128, 64] x rhs=xe[:, j, :] [128, 33] accumulates
#     S^T|counts [64, 33] directly (N=33 -> cheap).
#   AllReduce [64, 33] across 8 cores (28us fixed cost; overlapped with
#     pass-2 one-hot prebuilds).
#   stats: invc = 1/max(c,1), mu = S*invc, table [128, 33] = [mu-eps | invc]
#     replicated to rows 64:128 for the B-half pairing.
#   pass 2, per outer chunk of 15 A-tiles + 15 B-tiles: transposed one-hot
#     ht [128, 15*128] built at 4x DVE (TensorScalarPtr is_equal vs the
#     per-partition iota) from a broadcast-DMA'd label row; 3 matmuls per
#     tile accumulate  psum[:, slot, 0:32] = gather(mu-eps) - x  (the diff
#     computed entirely on PE via a -Identity matmul) and
#     psum[:, slot, 32] = gather(invc); Act squares the PSUM diff to bf16;
#     DVE reduces via log2 halving adds (packed bf16 -> 2x mode).
#   finals: dist = sqrt(d2); hinge = relu(dist-1.5); intra partial =
#     sum hinge^2 * invc via 2 muls + row reduce + partition reduce.
#   inter/reg (KxK) replicated on every core from the reduced stats.
import math
import numpy as np
from contextlib import ExitStack

import concourse.bass as bass
import concourse.bacc as bacc
import concourse.tile as tile
import concourse.mybir as mybir
from concourse.bass_utils import run_bass_kernel_spmd

F32 = mybir.dt.float32
BF16 = mybir.dt.bfloat16
I16 = mybir.dt.int16

N_CORES = 8
K = 64
D = 32
P = 128
EPS = 1e-8
PAD_LABEL = 999  # never matches any one-hot row (0..127)

INTRA_MARGIN = 1.5
INTER_MARGIN2 = 1.0  # 2 * 0.5

J1 = 30       # pass-1 tiles per one-hot chunk
JMG = 15      # pass-2 A-tiles (and B-tiles) per outer chunk
PREBUILD = 16   # pass-2 ht chunks emitted before the collective section
L2_BUFS = 6
HT_BUFS = 16


def _host_prep(features, labels, tpc):
    """Shard + relayout on host. Returns per-core input dicts."""
    n_total = features.shape[0]
    n_core = n_total // N_CORES
    n_pad = P * tpc
    import ml_dtypes

    na = (tpc + 1) // 2
    nout = math.ceil(na / JMG)
    iota_rep = np.tile(
        np.arange(K, dtype=np.int16)[None, :, None], (P, 1, J1)
    )
    iotacol = np.arange(P, dtype=np.float32).reshape(P, 1)
    negid = (-np.eye(P)).astype(ml_dtypes.bfloat16)
    id64 = np.eye(K, dtype=np.float32)
    eyeneg = (1.0 - np.eye(K, dtype=np.float32)).astype(ml_dtypes.bfloat16)

    in_maps = []
    for c in range(N_CORES):
        f = np.asarray(features[c * n_core : (c + 1) * n_core], dtype=np.float32)
        l = np.asarray(labels[c * n_core : (c + 1) * n_core], dtype=np.int64)
        if n_pad > n_core:
            f = np.concatenate([f, np.zeros((n_pad - n_core, D), np.float32)], axis=0)
            l = np.concatenate([l, np.full((n_pad - n_core,), PAD_LABEL, np.int64)])
        # xe: [P, tpc, 33] bf16, col 32 = 1.0
        xe = np.ones((n_pad, D + 1), np.float32)
        xe[:, :D] = f
        xe = xe.reshape(P, tpc, D + 1).astype(ml_dtypes.bfloat16)
        # p-major labels (pass-1 one-hot): [P, tpc] int16
        lpm = l.reshape(P, tpc).astype(np.int16)
        # tile-major labels for pass 2: ltm [nout, 2, JMG*P] int16,
        # [oc, 0] = A-tile labels, [oc, 1] = B-tile labels + 64.
        ltm_full = l.reshape(P, tpc).T.astype(np.int16)  # [tpc, P]
        ltm = np.full((nout, 2, JMG * P), PAD_LABEL, np.int16)
        for oc in range(nout):
            a0 = oc * JMG
            an = min(JMG, na - a0)
            ltm[oc, 0, : an * P] = ltm_full[a0 : a0 + an].ravel()
            b0 = na + a0
            bn = max(0, min(JMG, tpc - b0))
            if bn > 0:
                ltm[oc, 1, : bn * P] = ltm_full[b0 : b0 + bn].ravel() + 64
        in_maps.append(
            {
                "xe": np.ascontiguousarray(xe),
                "lpm": np.ascontiguousarray(lpm),
                "ltm": np.ascontiguousarray(ltm),
                "iota_rep": iota_rep,
                "iotacol": iotacol,
                "negid": negid,
                "id64": id64,
                "eyeneg": eyeneg,
            }
        )
    return in_maps


def build_program(tpc):
    """Build the SPMD Bass program. tpc = tiles per core (cols per partition)."""
    nc = bacc.Bacc(
        "TRN2", target_bir_lowering=False, debug=False, num_devices=N_CORES
    )
    core_ids = list(range(N_CORES))

    na = (tpc + 1) // 2
    nout = math.ceil(na / JMG)
    n_chunks1 = math.ceil(tpc / J1)

    xe_d = nc.dram_tensor("xe", [P, tpc, D + 1], BF16, kind="ExternalInput").ap()
    lpm_d = nc.dram_tensor("lpm", [P, tpc], I16, kind="ExternalInput").ap()
    ltm_d = nc.dram_tensor("ltm", [nout, 2, JMG * P], I16, kind="ExternalInput").ap()
    iota_rep_d = nc.dram_tensor("iota_rep", [P, K, J1], I16, kind="ExternalInput").ap()
    iotacol_d = nc.dram_tensor("iotacol", [P, 1], F32, kind="ExternalInput").ap()
    negid_d = nc.dram_tensor("negid", [P, P], BF16, kind="ExternalInput").ap()
    id64_d = nc.dram_tensor("id64", [K, K], F32, kind="ExternalInput").ap()
    eyeneg_d = nc.dram_tensor("eyeneg", [K, K], BF16, kind="ExternalInput").ap()
    out_d = nc.dram_tensor("out", [3], F32, kind="ExternalOutput").ap()

    with tile.TileContext(nc, num_cores=N_CORES) as tc, ExitStack() as ctx:
        singles = ctx.enter_context(tc.tile_pool(name="singles", bufs=1))
        xpool = ctx.enter_context(tc.tile_pool(name="xpool", bufs=1))
        hpool = ctx.enter_context(tc.tile_pool(name="hpool", bufs=2))
        l2pool = ctx.enter_context(tc.tile_pool(name="l2pool", bufs=L2_BUFS))
        htpool = ctx.enter_context(tc.tile_pool(name="htpool", bufs=HT_BUFS))
        sqpool = ctx.enter_context(tc.tile_pool(name="sqpool", bufs=2))
        hvpool = ctx.enter_context(tc.tile_pool(name="hvpool", bufs=2))
        wpool = ctx.enter_context(tc.tile_pool(name="wpool", bufs=2))
        psA = ctx.enter_context(tc.tile_pool(name="psA", bufs=1, space="PSUM"))
        psMg = ctx.enter_context(tc.tile_pool(name="psMg", bufs=3, space="PSUM"))
        psS = ctx.enter_context(tc.tile_pool(name="psS", bufs=1, space="PSUM"))
        dram = ctx.enter_context(tc.tile_pool(name="dram", bufs=2, space="DRAM"))

        # ---------- constants (critical first: pass-1 inputs) ----------
        lpm = singles.tile([P, tpc], I16)
        nc.sync.dma_start(out=lpm, in_=lpm_d)
        iota_rep = singles.tile([P, K, J1], I16)
        nc.sync.dma_start(out=iota_rep, in_=iota_rep_d)
        margneg = singles.tile([P, 1], F32)
        nc.vector.memset(margneg, -float(INTRA_MARGIN))
        ones64 = singles.tile([K, 1], F32)
        nc.vector.memset(ones64, 1.0)
        # prewarm the Act function table so the 1.3us load is off-critical
        actwarm = singles.tile([1, 1], F32)
        nc.scalar.activation(
            out=actwarm, in_=margneg[0:1, :],
            func=mybir.ActivationFunctionType.Square,
        )
        xe = xpool.tile([P, tpc, D + 1], BF16)

        d2all = singles.tile([P, tpc], F32)
        invc_all = singles.tile([P, tpc], BF16)

        # ---------- pass 1: segment sums (S^T | counts) [64, 33] ----------
        psumS = psA.tile([K, D + 1], F32)
        t_done = 0
        for c in range(n_chunks1):
            j0 = c * J1
            jn = min(J1, tpc - j0)
            nc.sync.dma_start(
                out=xe[:, j0 : j0 + jn, :], in_=xe_d[:, j0 : j0 + jn, :]
            )
            h2 = hpool.tile([P, K, J1], BF16, tag="h2")
            eng = nc.gpsimd if (c % 3 == 2) else nc.vector
            eng.tensor_tensor(
                h2[:, :, :jn],
                lpm[:, None, j0 : j0 + jn].to_broadcast((P, K, jn)),
                iota_rep[:, :, :jn],
                mybir.AluOpType.is_equal,
            )
            for j in range(jn):
                nc.tensor.matmul(
                    psumS,
                    h2[:, :, j],
                    xe[:, j0 + j, :],
                    start=(t_done == 0),
                    stop=(t_done == tpc - 1),
                )
                t_done += 1

        # ---------- remaining constants (needed only from pass 2 on) ----------
        iotacol = singles.tile([P, 1], F32)
        nc.sync.dma_start(out=iotacol, in_=iotacol_d)
        negid = singles.tile([P, P], BF16)
        nc.sync.dma_start(out=negid, in_=negid_d)
        id64 = singles.tile([K, K], F32)
        nc.sync.dma_start(out=id64, in_=id64_d)
        eyeneg = singles.tile([K, K], BF16)
        nc.sync.dma_start(out=eyeneg, in_=eyeneg_d)

        # ---------- pass-2 prep: prebuild label rows + transposed one-hots ----
        # (no dependency on the collective -> fills the AllReduce window)
        l2_tiles = {}
        ht_tiles = {}

        def emit_l2_ht(oc, eng=nc.vector):
            src = ltm_d[oc]
            l2 = l2pool.tile([P, JMG * P], I16, tag="l2")
            nc.sync.dma_start(
                out=l2,
                in_=bass.AP(
                    tensor=src.tensor,
                    offset=src.offset,
                    ap=[[JMG * P, 2], [0, K]] + [[1, JMG * P]],
                ),
            )
            ht = htpool.tile([P, JMG * P], BF16, tag="ht")
            eng.tensor_single_scalar(
                ht, l2, iotacol, mybir.AluOpType.is_equal
            )
            l2_tiles[oc] = l2
            ht_tiles[oc] = ht

        for oc in range(min(PREBUILD, nout)):
            emit_l2_ht(oc)

        # ---------- AllReduce the [64, 33] stats ----------
        sg_local = wpool.tile([K, D + 1], F32, tag="sg")
        nc.gpsimd.tensor_scalar_add(sg_local, psumS, 0.0)
        cc_in = dram.tile([K, D + 1], F32)
        cc_out = dram.tile([K, D + 1], F32)
        nc.gpsimd.dma_start(out=cc_in, in_=sg_local)
        nc.gpsimd.collective_compute(
            "AllReduce",
            mybir.AluOpType.add,
            replica_groups=[core_ids],
            ins=[cc_in.opt()],
            outs=[cc_out.opt()],
        )
        sg = singles.tile([K, D + 1], F32)
        nc.gpsimd.dma_start(out=sg, in_=cc_out)

        # ---------- critical stats: table = [mu - eps | invc] ----------
        safec = singles.tile([K, 1], F32)
        nc.vector.tensor_scalar_max(safec, sg[:, D : D + 1], 1.0)
        invc = singles.tile([K, 1], F32)
        nc.vector.reciprocal(invc, safec)
        mu = singles.tile([K, D], F32)
        nc.vector.tensor_mul(mu, sg[:, :D], invc.to_broadcast((K, D)))
        table = singles.tile([P, D + 1], BF16)
        nc.vector.tensor_scalar_add(table[:K, :D], mu, -EPS)
        nc.vector.tensor_scalar_add(table[:K, D : D + 1], invc, 0.0)
        # replicate rows [0,64) -> [64,128) for the B-half matmuls
        # (issued from the Pool queue, which is idle post-collective; keeping
        # it off SP avoids stalling the pass-2 label-row DMA stream)
        nc.gpsimd.dma_start(out=table[K:, :], in_=table[:K, :])

        # per-point finals tiles + helper (emitted in two segments so most of
        # the sqrt/relu/mul work overlaps pass 2)
        dist = singles.tile([P, tpc], BF16)
        hinge = singles.tile([P, tpc], BF16)
        hsq = singles.tile([P, tpc], BF16)
        hv = singles.tile([P, tpc], BF16)
        seg_done = (0, na)

        def emit_finals_segment(ca0, ca1, cb0, cb1):
            for c0, c1 in ((ca0, ca1), (cb0, cb1)):
                if c1 <= c0:
                    continue
                nc.scalar.activation(
                    out=dist[:, c0:c1], in_=d2all[:, c0:c1],
                    func=mybir.ActivationFunctionType.Sqrt,
                )
                nc.scalar.activation(
                    out=hinge[:, c0:c1], in_=dist[:, c0:c1],
                    func=mybir.ActivationFunctionType.Relu, bias=margneg,
                )
                nc.vector.tensor_mul(
                    hsq[:, c0:c1], hinge[:, c0:c1], hinge[:, c0:c1]
                )
                nc.vector.tensor_mul(
                    hv[:, c0:c1], hsq[:, c0:c1], invc_all[:, c0:c1]
                )

        # ---------- pass 2: diff via PE, square on Act, reduce on DVE ----------
        for oc in range(nout):
            a0 = oc * JMG
            an = min(JMG, na - a0)
            b0 = na + a0
            bn = max(0, min(JMG, tpc - b0))
            if oc not in ht_tiles:
                # 2 of 3 in-loop one-hot builds go to the (otherwise idle)
                # Pool engine; DVE keeps the rest plus the halving reduce
                eng = nc.vector if ((oc - PREBUILD) % 3 == 2) else nc.gpsimd
                emit_l2_ht(oc, eng)
            ht = ht_tiles.pop(oc)
            l2_tiles.pop(oc)

            # psmg [P, 32, 32] = exactly 2 psum banks; 32-wide diff slots never
            # cross a bank boundary. Slots 0:15 = A diffs, 15:30 = B diffs,
            # slot 30 col i = A-tile-i invc, slot 31 col i = B-tile-i invc.
            psmg = psMg.tile([P, 2 * JMG + 2, D], F32, tag="psmg")
            for i in range(an):
                hta = ht[:K, i * P : (i + 1) * P]
                # gather(mu-eps) opens the group; negid accumulates -x
                nc.tensor.matmul(
                    psmg[:, i, :], hta, table[:K, :D], start=True, stop=False
                )
                nc.tensor.matmul(
                    psmg[:, i, :], negid, xe[:, a0 + i, :D],
                    start=False, stop=True,
                )
                nc.tensor.matmul(
                    psmg[:, 2 * JMG, i : i + 1], hta, table[:K, D : D + 1],
                    start=True, stop=True,
                )
            for i in range(bn):
                htb = ht[K:, i * P : (i + 1) * P]
                s = JMG + i
                nc.tensor.matmul(
                    psmg[:, s, :], htb, table[K:, :D], start=True, stop=False
                )
                nc.tensor.matmul(
                    psmg[:, s, :], negid, xe[:, b0 + i, :D],
                    start=False, stop=True,
                )
                nc.tensor.matmul(
                    psmg[:, 2 * JMG + 1, i : i + 1], htb, table[K:, D : D + 1],
                    start=True, stop=True,
                )

            # invc gather extraction (Pool engine)
            nc.gpsimd.tensor_scalar_add(
                invc_all[:, a0 : a0 + an], psmg[:, 2 * JMG, :an], 0.0
            )
            if bn > 0:
                nc.gpsimd.tensor_scalar_add(
                    invc_all[:, b0 : b0 + bn], psmg[:, 2 * JMG + 1, :bn], 0.0
                )
            # square + halving-add reduction over D=32 (packed bf16 -> 2x DVE)
            sq = sqpool.tile([P, 2 * JMG, D], BF16, tag="sq")
            h16 = hvpool.tile([P, 2 * JMG, 16], BF16, tag="h16")
            h8 = hvpool.tile([P, 2 * JMG, 8], BF16, tag="h8")
            h4 = hvpool.tile([P, 2 * JMG, 4], BF16, tag="h4")
            h2t = hvpool.tile([P, 2 * JMG, 2], BF16, tag="h2t")
            if an == JMG and bn == JMG:
                ranges = [(0, 2 * JMG)]  # one combined pass over all 30 slots
            else:
                ranges = [(0, an)] + ([(JMG, bn)] if bn > 0 else [])
            for s0, n in ranges:
                if oc % 11 == 10:
                    # a few squares on DVE to keep Act off the critical path
                    nc.vector.tensor_mul(
                        sq[:, s0 : s0 + n, :],
                        psmg[:, s0 : s0 + n, :], psmg[:, s0 : s0 + n, :],
                    )
                else:
                    nc.scalar.activation(
                        out=sq[:, s0 : s0 + n, :], in_=psmg[:, s0 : s0 + n, :],
                        func=mybir.ActivationFunctionType.Square,
                    )
                nc.vector.tensor_add(
                    h16[:, s0 : s0 + n, :],
                    sq[:, s0 : s0 + n, 0:16], sq[:, s0 : s0 + n, 16:32],
                )
                nc.vector.tensor_add(
                    h8[:, s0 : s0 + n, :],
                    h16[:, s0 : s0 + n, 0:8], h16[:, s0 : s0 + n, 8:16],
                )
                nc.vector.tensor_add(
                    h4[:, s0 : s0 + n, :],
                    h8[:, s0 : s0 + n, 0:4], h8[:, s0 : s0 + n, 4:8],
                )
                nc.vector.tensor_add(
                    h2t[:, s0 : s0 + n, :],
                    h4[:, s0 : s0 + n, 0:2], h4[:, s0 : s0 + n, 2:4],
                )
            nc.vector.tensor_add(
                d2all[:, a0 : a0 + an], h2t[:, :an, 0], h2t[:, :an, 1]
            )
            if bn > 0:
                nc.vector.tensor_add(
                    d2all[:, b0 : b0 + bn],
                    h2t[:, JMG : JMG + bn, 0],
                    h2t[:, JMG : JMG + bn, 1],
                )
            if oc == nout // 2 - 1:
                # overlap the first half of the per-point finals with pass 2
                emit_finals_segment(0, a0 + an, na, b0 + bn)
                seg_done = (a0 + an, b0 + bn)

        # ---------- inter + reg (tiny, replicated) ----------
        mup = wpool.tile([K, D], F32, tag="mup")
        nc.vector.tensor_scalar_add(mup, mu, EPS)
        qsc = wpool.tile([K, D], F32, tag="qsc")
        nc.vector.tensor_mul(qsc, mu, mu)
        q = wpool.tile([K, 1], F32, tag="q")
        nc.vector.tensor_reduce(
            out=q, in_=qsc, axis=mybir.AxisListType.X, op=mybir.AluOpType.add
        )
        qpsc = wpool.tile([K, D], F32, tag="qpsc")
        nc.vector.tensor_mul(qpsc, mup, mup)
        qp = wpool.tile([K, 1], F32, tag="qp")
        nc.vector.tensor_reduce(
            out=qp, in_=qpsc, axis=mybir.AxisListType.X, op=mybir.AluOpType.add
        )
        # pd2[a,b] = qp_a - 2*mup_a.mu_b + q_b via one [64,64] matmul
        ab = wpool.tile([K, D + 2], F32, tag="ab")  # [-2*mup | qp | 1]
        nc.gpsimd.tensor_scalar_mul(ab[:, :D], mup, -2.0)
        nc.gpsimd.tensor_scalar_add(ab[:, D : D + 1], qp, 0.0)
        nc.vector.memset(ab[:, D + 1 : D + 2], 1.0)
        bb = wpool.tile([K, D + 2], F32, tag="bb")  # [mu | 1 | q]
        nc.gpsimd.tensor_scalar_add(bb[:, :D], mu, 0.0)
        nc.vector.memset(bb[:, D : D + 1], 1.0)
        nc.gpsimd.tensor_scalar_add(bb[:, D + 1 : D + 2], q, 0.0)
        psT = psS.tile([D + 2, K], F32, tag="small")
        nc.tensor.transpose(psT, ab, id64)
        atp = wpool.tile([D + 2, K], F32, tag="atp")
        nc.gpsimd.tensor_scalar_add(atp, psT, 0.0)
        psT2 = psS.tile([D + 2, K], F32, tag="small")
        nc.tensor.transpose(psT2, bb, id64)
        btp = wpool.tile([D + 2, K], F32, tag="btp")
        nc.gpsimd.tensor_scalar_add(btp, psT2, 0.0)
        psPD = psS.tile([K, K], F32, tag="small")
        nc.tensor.matmul(psPD, atp, btp)
        pdc = wpool.tile([K, K], F32, tag="pdc")
        nc.vector.tensor_scalar_max(pdc, psPD, 0.0)
        pdist = wpool.tile([K, K], F32, tag="pdist")
        nc.scalar.activation(
            out=pdist, in_=pdc, func=mybir.ActivationFunctionType.Sqrt
        )
        hingeI = wpool.tile([K, K], F32, tag="hingeI")
        nc.scalar.activation(
            out=hingeI, in_=pdist, func=mybir.ActivationFunctionType.Relu,
            bias=float(INTER_MARGIN2), scale=-1.0,
        )
        hm = wpool.tile([K, K], F32, tag="hm")
        nc.vector.tensor_mul(hm, hingeI, eyeneg)
        hm2 = wpool.tile([K, K], F32, tag="hm2")
        nc.vector.tensor_mul(hm2, hm, hm)
        interp = wpool.tile([K, 1], F32, tag="interp")
        nc.vector.tensor_reduce(
            out=interp, in_=hm2, axis=mybir.AxisListType.X, op=mybir.AluOpType.add
        )
        sqp = wpool.tile([K, 1], F32, tag="sqp")
        nc.scalar.activation(
            out=sqp, in_=qp, func=mybir.ActivationFunctionType.Sqrt
        )
        cat2 = wpool.tile([K, 2], F32, tag="cat2")
        nc.gpsimd.tensor_scalar_add(cat2[:, 0:1], interp, 0.0)
        nc.gpsimd.tensor_scalar_add(cat2[:, 1:2], sqp, 0.0)
        psIR = psS.tile([1, 2], F32, tag="small")
        nc.tensor.matmul(psIR, ones64, cat2)
        ir = wpool.tile([1, 2], F32, tag="ir")  # [inter_sum, reg_sum]
        nc.gpsimd.tensor_scalar_add(ir, psIR, 0.0)

        # ---------- intra finals (second segment + accumulation) ----------
        emit_finals_segment(seg_done[0], na, seg_done[1], tpc)
        rowsum = singles.tile([P, 1], F32)
        nc.vector.tensor_reduce(
            out=rowsum, in_=hv, axis=mybir.AxisListType.X,
            op=mybir.AluOpType.add,
        )
        intra = wpool.tile([1, 1], F32, tag="intra")
        nc.gpsimd.tensor_reduce(
            out=intra, in_=rowsum, axis=mybir.AxisListType.C,
            op=mybir.AluOpType.add,
        )
        tot = wpool.tile([1, 3], F32, tag="tot")
        nc.scalar.copy(out=tot[:, 0:1], in_=intra)
        nc.scalar.copy(out=tot[:, 1:3], in_=ir)
        nc.sync.dma_start(out=out_d, in_=tot[0:1, :])

    nc.compile()
    return nc


_NC_CACHE = {}


def _get_program(tpc):
    if tpc not in _NC_CACHE:
        _NC_CACHE[tpc] = build_program(tpc)
    return _NC_CACHE[tpc]


def kernel(features, labels, num_clusters):
    features = np.asarray(features)
    labels = np.asarray(labels)
    n_total = features.shape[0]
    n_core = n_total // N_CORES
    tpc = math.ceil(n_core / P)
    nc = _get_program(tpc)
    in_maps = _host_prep(features, labels, tpc)
    res = run_bass_kernel_spmd(nc, in_maps, list(range(N_CORES)))
    intra_sum = sum(float(res.results[c]["out"][0]) for c in range(N_CORES))
    inter_sum = float(res.results[0]["out"][1])
    reg_sum = float(res.results[0]["out"][2])
    total = (
        intra_sum / K
        + inter_sum / (K * (K - 1))
        + 0.001 * reg_sum / K
    )
    return np.float32(total)


# revision 18
# speedup vs baseline: 1.6554x; 1.0188x over previous
# kernel.py — DiscriminativeLoss on 8 TRN2 NeuronCores (Bass/Tile, SPMD).
#
# Math (matches reference):
#   counts_k = #{i: l_i = k};  S_k = sum_{i in k} x_i;  mu_k = S_k / max(c_k, 1)
#   intra = (1/K) * sum_i invc_{l_i} * relu(||x_i - mu_{l_i} + eps|| - 1.5)^2
#   inter = sum_{a != b} relu(1 - ||(mu_a + eps) - mu_b||)^2 / (K*(K-1))
#   reg   = (1/K) * sum_k ||mu_k + eps||
#   total = intra + inter + 0.001 * reg
#
# Device strategy (per core, data-parallel over points; point i = p*tpc + j
# lives at [p, j]):
#   pass 1: one-hot H2 [128, 64, J1] built per chunk via a single DVE
#     tensor_tensor is_equal against a materialized replicated iota (all
#     operands packed 2-byte -> 2x DVE mode); per-tile PE matmul
#     lhsT=H2[:, :, j] [128, 64] x rhs=xe[:, j, :] [128, 33] accumulates
#     S^T|counts [64, 33] directly (N=33 -> cheap).
#   AllReduce [64, 33] across 8 cores (28us fixed cost; overlapped with
#     pass-2 one-hot prebuilds).
#   stats: invc = 1/max(c,1), mu = S*invc, table [128, 33] = [mu-eps | invc]
#     replicated to rows 64:128 for the B-half pairing.
#   pass 2, per outer chunk of 15 A-tiles + 15 B-tiles: transposed one-hot
#     ht [128, 15*128] built at 4x DVE (TensorScalarPtr is_equal vs the
#     per-partition iota) from a broadcast-DMA'd label row; 3 matmuls per
#     tile accumulate  psum[:, slot, 0:32] = gather(mu-eps) - x  (the diff
#     computed entirely on PE via a -Identity matmul) and
#     psum[:, slot, 32] = gather(invc); Act squares the PSUM diff to bf16;
#     DVE reduces via log2 halving adds (packed bf16 -> 2x mode).
#   finals: dist = sqrt(d2); hinge = relu(dist-1.5); intra partial =
#     sum hinge^2 * invc via 2 muls + row reduce + partition reduce.
#   inter/reg (KxK) replicated on every core from the reduced stats.
import math
import numpy as np
from contextlib import ExitStack

import concourse.bass as bass
import concourse.bacc as bacc
import concourse.tile as tile
import concourse.mybir as mybir
from concourse.bass_utils import run_bass_kernel_spmd

F32 = mybir.dt.float32
BF16 = mybir.dt.bfloat16
I16 = mybir.dt.int16

N_CORES = 8
K = 64
D = 32
P = 128
EPS = 1e-8
PAD_LABEL = 999  # never matches any one-hot row (0..127)

INTRA_MARGIN = 1.5
INTER_MARGIN2 = 1.0  # 2 * 0.5

J1 = 30       # pass-1 tiles per one-hot chunk
JMG = 15      # pass-2 A-tiles (and B-tiles) per outer chunk
PREBUILD = 16   # pass-2 ht chunks emitted before the collective section
L2_BUFS = 6
HT_BUFS = 16


def _host_prep(features, labels, tpc):
    """Shard + relayout on host. Returns per-core input dicts."""
    n_total = features.shape[0]
    n_core = n_total // N_CORES
    n_pad = P * tpc
    import ml_dtypes

    na = (tpc + 1) // 2
    nout = math.ceil(na / JMG)
    iota_rep = np.tile(
        np.arange(K, dtype=np.int16)[None, :, None], (P, 1, J1)
    )
    iotacol = np.arange(P, dtype=np.float32).reshape(P, 1)
    negid = (-np.eye(P)).astype(ml_dtypes.bfloat16)
    id64 = np.eye(K, dtype=np.float32)
    eyeneg = (1.0 - np.eye(K, dtype=np.float32)).astype(ml_dtypes.bfloat16)

    in_maps = []
    for c in range(N_CORES):
        f = np.asarray(features[c * n_core : (c + 1) * n_core], dtype=np.float32)
        l = np.asarray(labels[c * n_core : (c + 1) * n_core], dtype=np.int64)
        if n_pad > n_core:
            f = np.concatenate([f, np.zeros((n_pad - n_core, D), np.float32)], axis=0)
            l = np.concatenate([l, np.full((n_pad - n_core,), PAD_LABEL, np.int64)])
        # xe: [P, tpc, 33] bf16, col 32 = 1.0
        xe = np.ones((n_pad, D + 1), np.float32)
        xe[:, :D] = f
        xe = xe.reshape(P, tpc, D + 1).astype(ml_dtypes.bfloat16)
        # p-major labels (pass-1 one-hot): [P, tpc] int16
        lpm = l.reshape(P, tpc).astype(np.int16)
        # tile-major labels for pass 2: ltm [nout, 2, JMG*P] int16,
        # [oc, 0] = A-tile labels, [oc, 1] = B-tile labels + 64.
        ltm_full = l.reshape(P, tpc).T.astype(np.int16)  # [tpc, P]
        ltm = np.full((nout, 2, JMG * P), PAD_LABEL, np.int16)
        for oc in range(nout):
            a0 = oc * JMG
            an = min(JMG, na - a0)
            ltm[oc, 0, : an * P] = ltm_full[a0 : a0 + an].ravel()
            b0 = na + a0
            bn = max(0, min(JMG, tpc - b0))
            if bn > 0:
                ltm[oc, 1, : bn * P] = ltm_full[b0 : b0 + bn].ravel() + 64
        in_maps.append(
            {
                "xe": np.ascontiguousarray(xe),
                "lpm": np.ascontiguousarray(lpm),
                "ltm": np.ascontiguousarray(ltm),
                "iota_rep": iota_rep,
                "iotacol": iotacol,
                "negid": negid,
                "id64": id64,
                "eyeneg": eyeneg,
            }
        )
    return in_maps


def build_program(tpc):
    """Build the SPMD Bass program. tpc = tiles per core (cols per partition)."""
    nc = bacc.Bacc(
        "TRN2", target_bir_lowering=False, debug=False, num_devices=N_CORES
    )
    core_ids = list(range(N_CORES))

    na = (tpc + 1) // 2
    nout = math.ceil(na / JMG)
    n_chunks1 = math.ceil(tpc / J1)

    xe_d = nc.dram_tensor("xe", [P, tpc, D + 1], BF16, kind="ExternalInput").ap()
    lpm_d = nc.dram_tensor("lpm", [P, tpc], I16, kind="ExternalInput").ap()
    ltm_d = nc.dram_tensor("ltm", [nout, 2, JMG * P], I16, kind="ExternalInput").ap()
    iota_rep_d = nc.dram_tensor("iota_rep", [P, K, J1], I16, kind="ExternalInput").ap()
    iotacol_d = nc.dram_tensor("iotacol", [P, 1], F32, kind="ExternalInput").ap()
    negid_d = nc.dram_tensor("negid", [P, P], BF16, kind="ExternalInput").ap()
    id64_d = nc.dram_tensor("id64", [K, K], F32, kind="ExternalInput").ap()
    eyeneg_d = nc.dram_tensor("eyeneg", [K, K], BF16, kind="ExternalInput").ap()
    out_d = nc.dram_tensor("out", [3], F32, kind="ExternalOutput").ap()

    with tile.TileContext(nc, num_cores=N_CORES) as tc, ExitStack() as ctx:
        singles = ctx.enter_context(tc.tile_pool(name="singles", bufs=1))
        xpool = ctx.enter_context(tc.tile_pool(name="xpool", bufs=1))
        hpool = ctx.enter_context(tc.tile_pool(name="hpool", bufs=4))
        l2pool = ctx.enter_context(tc.tile_pool(name="l2pool", bufs=L2_BUFS))
        htpool = ctx.enter_context(tc.tile_pool(name="htpool", bufs=HT_BUFS))
        sqpool = ctx.enter_context(tc.tile_pool(name="sqpool", bufs=2))
        hvpool = ctx.enter_context(tc.tile_pool(name="hvpool", bufs=2))
        wpool = ctx.enter_context(tc.tile_pool(name="wpool", bufs=2))
        psA = ctx.enter_context(tc.tile_pool(name="psA", bufs=1, space="PSUM"))
        psMg = ctx.enter_context(tc.tile_pool(name="psMg", bufs=3, space="PSUM"))
        psS = ctx.enter_context(tc.tile_pool(name="psS", bufs=1, space="PSUM"))
        dram = ctx.enter_context(tc.tile_pool(name="dram", bufs=2, space="DRAM"))

        # ---------- constants (critical first: pass-1 inputs) ----------
        lpm = singles.tile([P, tpc], I16)
        nc.sync.dma_start(out=lpm, in_=lpm_d)
        iota_rep = singles.tile([P, K, J1], I16)
        nc.sync.dma_start(out=iota_rep, in_=iota_rep_d)
        margneg = singles.tile([P, 1], F32)
        nc.vector.memset(margneg, -float(INTRA_MARGIN))
        ones64 = singles.tile([K, 1], F32)
        nc.vector.memset(ones64, 1.0)
        # prewarm the Act function table with Sqrt: narrows the possible
        # table sets to one containing sqrt+square+relu+copy, so the single
        # 1.3us load happens here, off-critical, and never again
        actwarm = singles.tile([1, 1], F32)
        nc.scalar.activation(
            out=actwarm, in_=ones64[0:1, :],
            func=mybir.ActivationFunctionType.Sqrt,
        )
        xe = xpool.tile([P, tpc, D + 1], BF16)

        d2all = singles.tile([P, tpc], F32)
        invc_all = singles.tile([P, tpc], BF16)

        # ---------- pass 1: segment sums (S^T | counts) [64, 33] ----------
        psumS = psA.tile([K, D + 1], F32)
        t_done = 0
        for c in range(n_chunks1):
            j0 = c * J1
            jn = min(J1, tpc - j0)
            nc.sync.dma_start(
                out=xe[:, j0 : j0 + jn, :], in_=xe_d[:, j0 : j0 + jn, :]
            )
            h2 = hpool.tile([P, K, J1], BF16, tag="h2")
            eng = nc.gpsimd if (c % 3 == 2) else nc.vector
            eng.tensor_tensor(
                h2[:, :, :jn],
                lpm[:, None, j0 : j0 + jn].to_broadcast((P, K, jn)),
                iota_rep[:, :, :jn],
                mybir.AluOpType.is_equal,
            )
            for j in range(jn):
                nc.tensor.matmul(
                    psumS,
                    h2[:, :, j],
                    xe[:, j0 + j, :],
                    start=(t_done == 0),
                    stop=(t_done == tpc - 1),
                )
                t_done += 1

        # ---------- remaining constants (needed only from pass 2 on) ----------
        iotacol = singles.tile([P, 1], F32)
        nc.sync.dma_start(out=iotacol, in_=iotacol_d)
        negid = singles.tile([P, P], BF16)
        nc.sync.dma_start(out=negid, in_=negid_d)
        id64 = singles.tile([K, K], F32)
        nc.sync.dma_start(out=id64, in_=id64_d)
        eyeneg = singles.tile([K, K], BF16)
        nc.sync.dma_start(out=eyeneg, in_=eyeneg_d)

        # ---------- pass-2 prep: prebuild label rows + transposed one-hots ----
        # (no dependency on the collective -> fills the AllReduce window)
        l2_tiles = {}
        ht_tiles = {}

        def emit_l2_ht(oc, eng=nc.vector):
            src = ltm_d[oc]
            l2 = l2pool.tile([P, JMG * P], I16, tag="l2")
            nc.sync.dma_start(
                out=l2,
                in_=bass.AP(
                    tensor=src.tensor,
                    offset=src.offset,
                    ap=[[JMG * P, 2], [0, K]] + [[1, JMG * P]],
                ),
            )
            ht = htpool.tile([P, JMG * P], BF16, tag="ht")
            eng.tensor_single_scalar(
                ht, l2, iotacol, mybir.AluOpType.is_equal
            )
            l2_tiles[oc] = l2
            ht_tiles[oc] = ht

        for oc in range(min(PREBUILD, nout)):
            emit_l2_ht(oc)

        # ---------- AllReduce the [64, 33] stats ----------
        sg_local = wpool.tile([K, D + 1], F32, tag="sg")
        nc.gpsimd.tensor_scalar_add(sg_local, psumS, 0.0)
        cc_in = dram.tile([K, D + 1], F32)
        cc_out = dram.tile([K, D + 1], F32)
        nc.gpsimd.dma_start(out=cc_in, in_=sg_local)
        nc.gpsimd.collective_compute(
            "AllReduce",
            mybir.AluOpType.add,
            replica_groups=[core_ids],
            ins=[cc_in.opt()],
            outs=[cc_out.opt()],
        )
        sg = singles.tile([K, D + 1], F32)
        nc.gpsimd.dma_start(out=sg, in_=cc_out)

        # ---------- critical stats: table = [mu - eps | invc] ----------
        safec = singles.tile([K, 1], F32)
        nc.vector.tensor_scalar_max(safec, sg[:, D : D + 1], 1.0)
        invc = singles.tile([K, 1], F32)
        nc.vector.reciprocal(invc, safec)
        mu = singles.tile([K, D], F32)
        nc.vector.tensor_mul(mu, sg[:, :D], invc.to_broadcast((K, D)))
        table = singles.tile([P, D + 1], BF16)
        nc.vector.tensor_scalar_add(table[:K, :D], mu, -EPS)
        nc.vector.tensor_scalar_add(table[:K, D : D + 1], invc, 0.0)
        # replicate rows [0,64) -> [64,128) for the B-half matmuls
        # (issued from the Pool queue, which is idle post-collective; keeping
        # it off SP avoids stalling the pass-2 label-row DMA stream)
        nc.gpsimd.dma_start(out=table[K:, :], in_=table[:K, :])

        # per-point finals tiles + helper (emitted in segments so most of
        # the sqrt/relu/mul/accumulate work overlaps pass 2). The running
        # per-partition intra sum is chained through tensor_tensor_reduce's
        # accumulator seed.
        dist = singles.tile([P, tpc], BF16)
        hinge = singles.tile([P, tpc], BF16)
        hsq = singles.tile([P, tpc], BF16)
        hv = singles.tile([P, tpc], BF16)
        seg_done = (0, na)
        racc_tiles = []

        def emit_finals_segment(ca0, ca1, cb0, cb1):
            for c0, c1 in ((ca0, ca1), (cb0, cb1)):
                if c1 <= c0:
                    continue
                nc.scalar.activation(
                    out=dist[:, c0:c1], in_=d2all[:, c0:c1],
                    func=mybir.ActivationFunctionType.Sqrt,
                )
                nc.scalar.activation(
                    out=hinge[:, c0:c1], in_=dist[:, c0:c1],
                    func=mybir.ActivationFunctionType.Relu, bias=margneg,
                )
                nc.vector.tensor_mul(
                    hsq[:, c0:c1], hinge[:, c0:c1], hinge[:, c0:c1]
                )
                racc = singles.tile([P, 1], F32, tag=f"racc{len(racc_tiles)}")
                nc.vector.tensor_tensor_reduce(
                    out=hv[:, c0:c1], in0=hsq[:, c0:c1],
                    in1=invc_all[:, c0:c1], scale=1.0,
                    scalar=(racc_tiles[-1] if racc_tiles else 0.0),
                    op0=mybir.AluOpType.mult, op1=mybir.AluOpType.add,
                    accum_out=racc,
                )
                racc_tiles.append(racc)

        # ---------- pass 2: diff via PE, square on Act, reduce on DVE ----------
        for oc in range(nout):
            a0 = oc * JMG
            an = min(JMG, na - a0)
            b0 = na + a0
            bn = max(0, min(JMG, tpc - b0))
            if oc not in ht_tiles:
                # in-loop one-hot builds go to the (otherwise idle) Pool
                # engine; DVE keeps the halving reduce
                emit_l2_ht(oc, nc.gpsimd)
            ht = ht_tiles.pop(oc)
            l2_tiles.pop(oc)

            # psmg [P, 32, 32] = exactly 2 psum banks; 32-wide diff slots never
            # cross a bank boundary. Slots 0:15 = A diffs, 15:30 = B diffs,
            # slot 30 col i = A-tile-i invc, slot 31 col i = B-tile-i invc.
            psmg = psMg.tile([P, 2 * JMG + 2, D], F32, tag="psmg")
            for i in range(an):
                hta = ht[:K, i * P : (i + 1) * P]
                # gather(mu-eps) opens the group; negid accumulates -x
                nc.tensor.matmul(
                    psmg[:, i, :], hta, table[:K, :D], start=True, stop=False
                )
                nc.tensor.matmul(
                    psmg[:, i, :], negid, xe[:, a0 + i, :D],
                    start=False, stop=True,
                )
                nc.tensor.matmul(
                    psmg[:, 2 * JMG, i : i + 1], hta, table[:K, D : D + 1],
                    start=True, stop=True,
                )
            for i in range(bn):
                htb = ht[K:, i * P : (i + 1) * P]
                s = JMG + i
                nc.tensor.matmul(
                    psmg[:, s, :], htb, table[K:, :D], start=True, stop=False
                )
                nc.tensor.matmul(
                    psmg[:, s, :], negid, xe[:, b0 + i, :D],
                    start=False, stop=True,
                )
                nc.tensor.matmul(
                    psmg[:, 2 * JMG + 1, i : i + 1], htb, table[K:, D : D + 1],
                    start=True, stop=True,
                )

            # invc gather extraction (Pool engine)
            nc.gpsimd.tensor_scalar_add(
                invc_all[:, a0 : a0 + an], psmg[:, 2 * JMG, :an], 0.0
            )
            if bn > 0:
                nc.gpsimd.tensor_scalar_add(
                    invc_all[:, b0 : b0 + bn], psmg[:, 2 * JMG + 1, :bn], 0.0
                )
            # square + halving-add reduction over D=32 (packed bf16 -> 2x DVE)
            sq = sqpool.tile([P, 2 * JMG, D], BF16, tag="sq")
            h16 = hvpool.tile([P, 2 * JMG, 16], BF16, tag="h16")
            h8 = hvpool.tile([P, 2 * JMG, 8], BF16, tag="h8")
            h4 = hvpool.tile([P, 2 * JMG, 4], BF16, tag="h4")
            h2t = hvpool.tile([P, 2 * JMG, 2], BF16, tag="h2t")
            if an == JMG and bn == JMG:
                ranges = [(0, 2 * JMG)]  # one combined pass over all 30 slots
            else:
                ranges = [(0, an)] + ([(JMG, bn)] if bn > 0 else [])
            for s0, n in ranges:
                if oc % 11 == 10:
                    # a few squares on DVE to keep Act off the critical path
                    nc.vector.tensor_mul(
                        sq[:, s0 : s0 + n, :],
                        psmg[:, s0 : s0 + n, :], psmg[:, s0 : s0 + n, :],
                    )
                else:
                    nc.scalar.activation(
                        out=sq[:, s0 : s0 + n, :], in_=psmg[:, s0 : s0 + n, :],
                        func=mybir.ActivationFunctionType.Square,
                    )
                nc.vector.tensor_add(
                    h16[:, s0 : s0 + n, :],
                    sq[:, s0 : s0 + n, 0:16], sq[:, s0 : s0 + n, 16:32],
                )
                nc.vector.tensor_add(
                    h8[:, s0 : s0 + n, :],
                    h16[:, s0 : s0 + n, 0:8], h16[:, s0 : s0 + n, 8:16],
                )
                nc.vector.tensor_add(
                    h4[:, s0 : s0 + n, :],
                    h8[:, s0 : s0 + n, 0:4], h8[:, s0 : s0 + n, 4:8],
                )
                nc.vector.tensor_add(
                    h2t[:, s0 : s0 + n, :],
                    h4[:, s0 : s0 + n, 0:2], h4[:, s0 : s0 + n, 2:4],
                )
            nc.vector.tensor_add(
                d2all[:, a0 : a0 + an], h2t[:, :an, 0], h2t[:, :an, 1]
            )
            if bn > 0:
                nc.vector.tensor_add(
                    d2all[:, b0 : b0 + bn],
                    h2t[:, JMG : JMG + bn, 0],
                    h2t[:, JMG : JMG + bn, 1],
                )
            if oc in (nout // 2 - 1, (3 * nout) // 4 - 1):
                # overlap most of the per-point finals with pass 2
                emit_finals_segment(seg_done[0], a0 + an, seg_done[1], b0 + bn)
                seg_done = (a0 + an, b0 + bn)

        # ---------- inter + reg (tiny, replicated) ----------
        mup = wpool.tile([K, D], F32, tag="mup")
        nc.vector.tensor_scalar_add(mup, mu, EPS)
        qsc = wpool.tile([K, D], F32, tag="qsc")
        nc.vector.tensor_mul(qsc, mu, mu)
        q = wpool.tile([K, 1], F32, tag="q")
        nc.vector.tensor_reduce(
            out=q, in_=qsc, axis=mybir.AxisListType.X, op=mybir.AluOpType.add
        )
        qpsc = wpool.tile([K, D], F32, tag="qpsc")
        nc.vector.tensor_mul(qpsc, mup, mup)
        qp = wpool.tile([K, 1], F32, tag="qp")
        nc.vector.tensor_reduce(
            out=qp, in_=qpsc, axis=mybir.AxisListType.X, op=mybir.AluOpType.add
        )
        # pd2[a,b] = qp_a - 2*mup_a.mu_b + q_b via one [64,64] matmul
        ab = wpool.tile([K, D + 2], F32, tag="ab")  # [-2*mup | qp | 1]
        nc.gpsimd.tensor_scalar_mul(ab[:, :D], mup, -2.0)
        nc.gpsimd.tensor_scalar_add(ab[:, D : D + 1], qp, 0.0)
        nc.vector.memset(ab[:, D + 1 : D + 2], 1.0)
        bb = wpool.tile([K, D + 2], F32, tag="bb")  # [mu | 1 | q]
        nc.gpsimd.tensor_scalar_add(bb[:, :D], mu, 0.0)
        nc.vector.memset(bb[:, D : D + 1], 1.0)
        nc.gpsimd.tensor_scalar_add(bb[:, D + 1 : D + 2], q, 0.0)
        psT = psS.tile([D + 2, K], F32, tag="small")
        nc.tensor.transpose(psT, ab, id64)
        atp = wpool.tile([D + 2, K], F32, tag="atp")
        nc.gpsimd.tensor_scalar_add(atp, psT, 0.0)
        psT2 = psS.tile([D + 2, K], F32, tag="small")
        nc.tensor.transpose(psT2, bb, id64)
        btp = wpool.tile([D + 2, K], F32, tag="btp")
        nc.gpsimd.tensor_scalar_add(btp, psT2, 0.0)
        psPD = psS.tile([K, K], F32, tag="small")
        nc.tensor.matmul(psPD, atp, btp)
        pdc = wpool.tile([K, K], F32, tag="pdc")
        nc.vector.tensor_scalar_max(pdc, psPD, 0.0)
        pdist = wpool.tile([K, K], F32, tag="pdist")
        nc.scalar.activation(
            out=pdist, in_=pdc, func=mybir.ActivationFunctionType.Sqrt
        )
        hingeI = wpool.tile([K, K], F32, tag="hingeI")
        nc.scalar.activation(
            out=hingeI, in_=pdist, func=mybir.ActivationFunctionType.Relu,
            bias=float(INTER_MARGIN2), scale=-1.0,
        )
        hm = wpool.tile([K, K], F32, tag="hm")
        nc.vector.tensor_mul(hm, hingeI, eyeneg)
        hm2 = wpool.tile([K, K], F32, tag="hm2")
        nc.vector.tensor_mul(hm2, hm, hm)
        interp = wpool.tile([K, 1], F32, tag="interp")
        nc.vector.tensor_reduce(
            out=interp, in_=hm2, axis=mybir.AxisListType.X, op=mybir.AluOpType.add
        )
        sqp = wpool.tile([K, 1], F32, tag="sqp")
        nc.scalar.activation(
            out=sqp, in_=qp, func=mybir.ActivationFunctionType.Sqrt
        )
        cat2 = wpool.tile([K, 2], F32, tag="cat2")
        nc.gpsimd.tensor_scalar_add(cat2[:, 0:1], interp, 0.0)
        nc.gpsimd.tensor_scalar_add(cat2[:, 1:2], sqp, 0.0)
        psIR = psS.tile([1, 2], F32, tag="small")
        nc.tensor.matmul(psIR, ones64, cat2)
        ir = wpool.tile([1, 2], F32, tag="ir")  # [inter_sum, reg_sum]
        nc.gpsimd.tensor_scalar_add(ir, psIR, 0.0)

        # ---------- intra finals (last segment + accumulation) ----------
        emit_finals_segment(seg_done[0], na, seg_done[1], tpc)
        intra = wpool.tile([1, 1], F32, tag="intra")
        nc.gpsimd.tensor_reduce(
            out=intra, in_=racc_tiles[-1], axis=mybir.AxisListType.C,
            op=mybir.AluOpType.add,
        )
        tot = wpool.tile([1, 3], F32, tag="tot")
        nc.scalar.copy(out=tot[:, 0:1], in_=intra)
        nc.scalar.copy(out=tot[:, 1:3], in_=ir)
        nc.sync.dma_start(out=out_d, in_=tot[0:1, :])

    nc.compile()
    return nc


_NC_CACHE = {}


def _get_program(tpc):
    if tpc not in _NC_CACHE:
        _NC_CACHE[tpc] = build_program(tpc)
    return _NC_CACHE[tpc]


def kernel(features, labels, num_clusters):
    features = np.asarray(features)
    labels = np.asarray(labels)
    n_total = features.shape[0]
    n_core = n_total // N_CORES
    tpc = math.ceil(n_core / P)
    nc = _get_program(tpc)
    in_maps = _host_prep(features, labels, tpc)
    res = run_bass_kernel_spmd(nc, in_maps, list(range(N_CORES)))
    intra_sum = sum(float(res.results[c]["out"][0]) for c in range(N_CORES))
    inter_sum = float(res.results[0]["out"][1])
    reg_sum = float(res.results[0]["out"][2])
    total = (
        intra_sum / K
        + inter_sum / (K * (K - 1))
        + 0.001 * reg_sum / K
    )
    return np.float32(total)


# revision 19
# speedup vs baseline: 1.6858x; 1.0184x over previous
# kernel.py — DiscriminativeLoss on 8 TRN2 NeuronCores (Bass/Tile, SPMD).
#
# Math (matches reference):
#   counts_k = #{i: l_i = k};  S_k = sum_{i in k} x_i;  mu_k = S_k / max(c_k, 1)
#   intra = (1/K) * sum_i invc_{l_i} * relu(||x_i - mu_{l_i} + eps|| - 1.5)^2
#   inter = sum_{a != b} relu(1 - ||(mu_a + eps) - mu_b||)^2 / (K*(K-1))
#   reg   = (1/K) * sum_k ||mu_k + eps||
#   total = intra + inter + 0.001 * reg
#
# Device strategy (per core, data-parallel over points; point i = p*tpc + j
# lives at [p, j]):
#   pass 1: one-hot H2 [128, 64, J1] built per chunk via a single DVE
#     tensor_tensor is_equal against a materialized replicated iota (all
#     operands packed 2-byte -> 2x DVE mode); per-tile PE matmul
#     lhsT=H2[:, :, j] [128, 64] x rhs=xe[:, j, :] [128, 33] accumulates
#     S^T|counts [64, 33] directly (N=33 -> cheap).
#   AllReduce [64, 33] across 8 cores (28us fixed cost; overlapped with
#     pass-2 one-hot prebuilds).
#   stats: invc = 1/max(c,1), mu = S*invc, table [128, 33] = [mu-eps | invc]
#     replicated to rows 64:128 for the B-half pairing.
#   pass 2, per outer chunk of 15 A-tiles + 15 B-tiles: transposed one-hot
#     ht [128, 15*128] built at 4x DVE (TensorScalarPtr is_equal vs the
#     per-partition iota) from a broadcast-DMA'd label row; 3 matmuls per
#     tile accumulate  psum[:, slot, 0:32] = gather(mu-eps) - x  (the diff
#     computed entirely on PE via a -Identity matmul) and
#     psum[:, slot, 32] = gather(invc); Act squares the PSUM diff to bf16;
#     DVE reduces via log2 halving adds (packed bf16 -> 2x mode).
#   finals: dist = sqrt(d2); hinge = relu(dist-1.5); intra partial =
#     sum hinge^2 * invc via 2 muls + row reduce + partition reduce.
#   inter/reg (KxK) replicated on every core from the reduced stats.
import math
import numpy as np
from contextlib import ExitStack

import concourse.bass as bass
import concourse.bacc as bacc
import concourse.tile as tile
import concourse.mybir as mybir
from concourse.bass_utils import run_bass_kernel_spmd

F32 = mybir.dt.float32
BF16 = mybir.dt.bfloat16
I16 = mybir.dt.int16

N_CORES = 8
K = 64
D = 32
P = 128
EPS = 1e-8
PAD_LABEL = 999  # never matches any one-hot row (0..127)

INTRA_MARGIN = 1.5
INTER_MARGIN2 = 1.0  # 2 * 0.5

J1 = 30       # pass-1 tiles per one-hot chunk
JMG = 15      # pass-2 A-tiles (and B-tiles) per outer chunk
PREBUILD = 21   # pass-2 ht chunks emitted before the collective section
L2_BUFS = 6
HT_BUFS = 21


def _host_prep(features, labels, tpc):
    """Shard + relayout on host. Returns per-core input dicts."""
    n_total = features.shape[0]
    n_core = n_total // N_CORES
    n_pad = P * tpc
    import ml_dtypes

    na = (tpc + 1) // 2
    nout = math.ceil(na / JMG)
    iota_rep = np.tile(
        np.arange(K, dtype=np.int16)[None, :, None], (P, 1, J1)
    )
    iotacol = np.arange(P, dtype=np.float32).reshape(P, 1)
    negid = (-np.eye(P)).astype(ml_dtypes.bfloat16)
    id64 = np.eye(K, dtype=np.float32)
    eyeneg = (1.0 - np.eye(K, dtype=np.float32)).astype(ml_dtypes.bfloat16)

    in_maps = []
    for c in range(N_CORES):
        f = np.asarray(features[c * n_core : (c + 1) * n_core], dtype=np.float32)
        l = np.asarray(labels[c * n_core : (c + 1) * n_core], dtype=np.int64)
        if n_pad > n_core:
            f = np.concatenate([f, np.zeros((n_pad - n_core, D), np.float32)], axis=0)
            l = np.concatenate([l, np.full((n_pad - n_core,), PAD_LABEL, np.int64)])
        # xe: [P, tpc, 32] bf16 (counts come from separate ones-rhs matmuls)
        xe = f.reshape(P, tpc, D).astype(ml_dtypes.bfloat16)
        # p-major labels (pass-1 one-hot): [P, tpc] int16
        lpm = l.reshape(P, tpc).astype(np.int16)
        # tile-major labels for pass 2: ltm [nout, 2, JMG*P] int16,
        # [oc, 0] = A-tile labels, [oc, 1] = B-tile labels + 64.
        ltm_full = l.reshape(P, tpc).T.astype(np.int16)  # [tpc, P]
        ltm = np.full((nout, 2, JMG * P), PAD_LABEL, np.int16)
        for oc in range(nout):
            a0 = oc * JMG
            an = min(JMG, na - a0)
            ltm[oc, 0, : an * P] = ltm_full[a0 : a0 + an].ravel()
            b0 = na + a0
            bn = max(0, min(JMG, tpc - b0))
            if bn > 0:
                ltm[oc, 1, : bn * P] = ltm_full[b0 : b0 + bn].ravel() + 64
        in_maps.append(
            {
                "xe": np.ascontiguousarray(xe),
                "lpm": np.ascontiguousarray(lpm),
                "ltm": np.ascontiguousarray(ltm),
                "iota_rep": iota_rep,
                "iotacol": iotacol,
                "negid": negid,
                "id64": id64,
                "eyeneg": eyeneg,
            }
        )
    return in_maps


def build_program(tpc):
    """Build the SPMD Bass program. tpc = tiles per core (cols per partition)."""
    nc = bacc.Bacc(
        "TRN2", target_bir_lowering=False, debug=False, num_devices=N_CORES
    )
    core_ids = list(range(N_CORES))

    na = (tpc + 1) // 2
    nout = math.ceil(na / JMG)
    n_chunks1 = math.ceil(tpc / J1)

    xe_d = nc.dram_tensor("xe", [P, tpc, D], BF16, kind="ExternalInput").ap()
    lpm_d = nc.dram_tensor("lpm", [P, tpc], I16, kind="ExternalInput").ap()
    ltm_d = nc.dram_tensor("ltm", [nout, 2, JMG * P], I16, kind="ExternalInput").ap()
    iota_rep_d = nc.dram_tensor("iota_rep", [P, K, J1], I16, kind="ExternalInput").ap()
    iotacol_d = nc.dram_tensor("iotacol", [P, 1], F32, kind="ExternalInput").ap()
    negid_d = nc.dram_tensor("negid", [P, P], BF16, kind="ExternalInput").ap()
    id64_d = nc.dram_tensor("id64", [K, K], F32, kind="ExternalInput").ap()
    eyeneg_d = nc.dram_tensor("eyeneg", [K, K], BF16, kind="ExternalInput").ap()
    out_d = nc.dram_tensor("out", [3], F32, kind="ExternalOutput").ap()

    with tile.TileContext(nc, num_cores=N_CORES) as tc, ExitStack() as ctx:
        singles = ctx.enter_context(tc.tile_pool(name="singles", bufs=1))
        xpool = ctx.enter_context(tc.tile_pool(name="xpool", bufs=1))
        hpool = ctx.enter_context(tc.tile_pool(name="hpool", bufs=3))
        l2pool = ctx.enter_context(tc.tile_pool(name="l2pool", bufs=L2_BUFS))
        htpool = ctx.enter_context(tc.tile_pool(name="htpool", bufs=HT_BUFS))
        sqpool = ctx.enter_context(tc.tile_pool(name="sqpool", bufs=3))
        hvpool = ctx.enter_context(tc.tile_pool(name="hvpool", bufs=2))
        wpool = ctx.enter_context(tc.tile_pool(name="wpool", bufs=2))
        psA = ctx.enter_context(tc.tile_pool(name="psA", bufs=1, space="PSUM"))
        psMg = ctx.enter_context(tc.tile_pool(name="psMg", bufs=3, space="PSUM"))
        psS = ctx.enter_context(tc.tile_pool(name="psS", bufs=1, space="PSUM"))
        dram = ctx.enter_context(tc.tile_pool(name="dram", bufs=2, space="DRAM"))

        # ---------- constants (critical first: pass-1 inputs) ----------
        lpm = singles.tile([P, tpc], I16)
        nc.sync.dma_start(out=lpm, in_=lpm_d)
        iota_rep = singles.tile([P, K, J1], I16)
        nc.sync.dma_start(out=iota_rep, in_=iota_rep_d)
        margneg = singles.tile([P, 1], F32)
        nc.vector.memset(margneg, -float(INTRA_MARGIN))
        ones64 = singles.tile([K, 1], F32)
        nc.vector.memset(ones64, 1.0)
        # prewarm the Act function table with Sqrt: narrows the possible
        # table sets to one containing sqrt+square+relu+copy, so the single
        # 1.3us load happens here, off-critical, and never again
        actwarm = singles.tile([1, 1], F32)
        nc.scalar.activation(
            out=actwarm, in_=ones64[0:1, :],
            func=mybir.ActivationFunctionType.Sqrt,
        )
        xe = xpool.tile([P, tpc, D], BF16)
        ones128 = singles.tile([P, 1], BF16)
        nc.vector.memset(ones128, 1.0)

        d2all = singles.tile([P, tpc], BF16)
        invc_all = singles.tile([P, tpc], BF16)

        # ---------- pass 1: segment sums S^T [64, 32] + counts [64, 1] ----------
        psumS = psA.tile([K, D], F32)
        psumC = psS.tile([K, 1], F32, tag="small")
        t_done = 0
        for c in range(n_chunks1):
            j0 = c * J1
            jn = min(J1, tpc - j0)
            nc.sync.dma_start(
                out=xe[:, j0 : j0 + jn, :], in_=xe_d[:, j0 : j0 + jn, :]
            )
            h2 = hpool.tile([P, K, J1], BF16, tag="h2")
            eng = nc.gpsimd if (c % 3 == 2) else nc.vector
            eng.tensor_tensor(
                h2[:, :, :jn],
                lpm[:, None, j0 : j0 + jn].to_broadcast((P, K, jn)),
                iota_rep[:, :, :jn],
                mybir.AluOpType.is_equal,
            )
            for j in range(jn):
                nc.tensor.matmul(
                    psumS,
                    h2[:, :, j],
                    xe[:, j0 + j, :],
                    start=(t_done == 0),
                    stop=(t_done == tpc - 1),
                )
                nc.tensor.matmul(
                    psumC,
                    h2[:, :, j],
                    ones128,
                    start=(t_done == 0),
                    stop=(t_done == tpc - 1),
                )
                t_done += 1

        # ---------- remaining constants (needed only from pass 2 on) ----------
        iotacol = singles.tile([P, 1], F32)
        nc.sync.dma_start(out=iotacol, in_=iotacol_d)
        negid = singles.tile([P, P], BF16)
        nc.sync.dma_start(out=negid, in_=negid_d)
        id64 = singles.tile([K, K], F32)
        nc.sync.dma_start(out=id64, in_=id64_d)
        eyeneg = singles.tile([K, K], BF16)
        nc.sync.dma_start(out=eyeneg, in_=eyeneg_d)

        # ---------- pass-2 prep: prebuild label rows + transposed one-hots ----
        # (no dependency on the collective -> fills the AllReduce window)
        l2_tiles = {}
        ht_tiles = {}

        def emit_l2_ht(oc, eng=nc.vector):
            src = ltm_d[oc]
            l2 = l2pool.tile([P, JMG * P], I16, tag="l2")
            nc.sync.dma_start(
                out=l2,
                in_=bass.AP(
                    tensor=src.tensor,
                    offset=src.offset,
                    ap=[[JMG * P, 2], [0, K]] + [[1, JMG * P]],
                ),
            )
            ht = htpool.tile([P, JMG * P], BF16, tag="ht")
            eng.tensor_single_scalar(
                ht, l2, iotacol, mybir.AluOpType.is_equal
            )
            l2_tiles[oc] = l2
            ht_tiles[oc] = ht

        for oc in range(min(PREBUILD, nout)):
            emit_l2_ht(oc)

        # ---------- AllReduce the [64, 33] stats ----------
        sg_local = wpool.tile([K, D + 1], F32, tag="sg")
        nc.gpsimd.tensor_scalar_add(sg_local[:, :D], psumS, 0.0)
        nc.gpsimd.tensor_scalar_add(sg_local[:, D : D + 1], psumC, 0.0)
        cc_in = dram.tile([K, D + 1], F32)
        cc_out = dram.tile([K, D + 1], F32)
        nc.gpsimd.dma_start(out=cc_in, in_=sg_local)
        nc.gpsimd.collective_compute(
            "AllReduce",
            mybir.AluOpType.add,
            replica_groups=[core_ids],
            ins=[cc_in.opt()],
            outs=[cc_out.opt()],
        )
        sg = singles.tile([K, D + 1], F32)
        nc.gpsimd.dma_start(out=sg, in_=cc_out)

        # ---------- critical stats: table = [mu - eps | invc] ----------
        safec = singles.tile([K, 1], F32)
        nc.vector.tensor_scalar_max(safec, sg[:, D : D + 1], 1.0)
        invc = singles.tile([K, 1], F32)
        nc.vector.reciprocal(invc, safec)
        mu = singles.tile([K, D], F32)
        nc.vector.tensor_mul(mu, sg[:, :D], invc.to_broadcast((K, D)))
        table = singles.tile([P, D + 1], BF16)
        nc.vector.tensor_scalar_add(table[:K, :D], mu, -EPS)
        nc.vector.tensor_scalar_add(table[:K, D : D + 1], invc, 0.0)
        # replicate rows [0,64) -> [64,128) for the B-half matmuls
        # (issued from the Pool queue, which is idle post-collective; keeping
        # it off SP avoids stalling the pass-2 label-row DMA stream)
        nc.gpsimd.dma_start(out=table[K:, :], in_=table[:K, :])

        # per-point finals tiles + helper (emitted in segments so most of
        # the sqrt/relu/mul/accumulate work overlaps pass 2). The running
        # per-partition intra sum is chained through tensor_tensor_reduce's
        # accumulator seed.
        dist = singles.tile([P, tpc], BF16)
        hinge = singles.tile([P, tpc], BF16)
        hsq = singles.tile([P, tpc], BF16)
        hv = singles.tile([P, tpc], BF16)
        seg_done = (0, na)
        racc_tiles = []

        def emit_finals_segment(ca0, ca1, cb0, cb1):
            for c0, c1 in ((ca0, ca1), (cb0, cb1)):
                if c1 <= c0:
                    continue
                nc.scalar.activation(
                    out=dist[:, c0:c1], in_=d2all[:, c0:c1],
                    func=mybir.ActivationFunctionType.Sqrt,
                )
                nc.scalar.activation(
                    out=hinge[:, c0:c1], in_=dist[:, c0:c1],
                    func=mybir.ActivationFunctionType.Relu, bias=margneg,
                )
                nc.vector.tensor_mul(
                    hsq[:, c0:c1], hinge[:, c0:c1], hinge[:, c0:c1]
                )
                racc = singles.tile([P, 1], F32, tag=f"racc{len(racc_tiles)}")
                nc.vector.tensor_tensor_reduce(
                    out=hv[:, c0:c1], in0=hsq[:, c0:c1],
                    in1=invc_all[:, c0:c1], scale=1.0,
                    scalar=(racc_tiles[-1] if racc_tiles else 0.0),
                    op0=mybir.AluOpType.mult, op1=mybir.AluOpType.add,
                    accum_out=racc,
                )
                racc_tiles.append(racc)

        # ---------- pass 2: diff via PE, square on Act, reduce on DVE ----------
        for oc in range(nout):
            a0 = oc * JMG
            an = min(JMG, na - a0)
            b0 = na + a0
            bn = max(0, min(JMG, tpc - b0))
            if oc not in ht_tiles:
                # in-loop one-hot builds go to the (otherwise idle) Pool
                # engine; DVE keeps the halving reduce
                emit_l2_ht(oc, nc.gpsimd)
            ht = ht_tiles.pop(oc)
            l2_tiles.pop(oc)

            # psmg [P, 32, 32] = exactly 2 psum banks; 32-wide diff slots never
            # cross a bank boundary. Slots 0:15 = A diffs, 15:30 = B diffs,
            # slot 30 col i = A-tile-i invc, slot 31 col i = B-tile-i invc.
            psmg = psMg.tile([P, 2 * JMG + 2, D], F32, tag="psmg")
            for i in range(an):
                hta = ht[:K, i * P : (i + 1) * P]
                # gather(mu-eps) opens the group; negid accumulates -x
                nc.tensor.matmul(
                    psmg[:, i, :], hta, table[:K, :D], start=True, stop=False
                )
                nc.tensor.matmul(
                    psmg[:, i, :], negid, xe[:, a0 + i, :],
                    start=False, stop=True,
                )
                nc.tensor.matmul(
                    psmg[:, 2 * JMG, i : i + 1], hta, table[:K, D : D + 1],
                    start=True, stop=True,
                )
            for i in range(bn):
                htb = ht[K:, i * P : (i + 1) * P]
                s = JMG + i
                nc.tensor.matmul(
                    psmg[:, s, :], htb, table[K:, :D], start=True, stop=False
                )
                nc.tensor.matmul(
                    psmg[:, s, :], negid, xe[:, b0 + i, :],
                    start=False, stop=True,
                )
                nc.tensor.matmul(
                    psmg[:, 2 * JMG + 1, i : i + 1], htb, table[K:, D : D + 1],
                    start=True, stop=True,
                )

            # invc gather extraction (Pool engine)
            nc.gpsimd.tensor_scalar_add(
                invc_all[:, a0 : a0 + an], psmg[:, 2 * JMG, :an], 0.0
            )
            if bn > 0:
                nc.gpsimd.tensor_scalar_add(
                    invc_all[:, b0 : b0 + bn], psmg[:, 2 * JMG + 1, :bn], 0.0
                )
            # square + halving-add reduction over D=32 (packed bf16 -> 2x DVE)
            sq = sqpool.tile([P, 2 * JMG, D], BF16, tag="sq")
            h16 = hvpool.tile([P, 2 * JMG, 16], BF16, tag="h16")
            h8 = hvpool.tile([P, 2 * JMG, 8], BF16, tag="h8")
            h4 = hvpool.tile([P, 2 * JMG, 4], BF16, tag="h4")
            h2t = hvpool.tile([P, 2 * JMG, 2], BF16, tag="h2t")
            if an == JMG and bn == JMG:
                ranges = [(0, 2 * JMG)]  # one combined pass over all 30 slots
            else:
                ranges = [(0, an)] + ([(JMG, bn)] if bn > 0 else [])
            for s0, n in ranges:
                if oc % 8 == 7:
                    # a few squares on DVE to keep Act off the critical path
                    nc.vector.tensor_mul(
                        sq[:, s0 : s0 + n, :],
                        psmg[:, s0 : s0 + n, :], psmg[:, s0 : s0 + n, :],
                    )
                else:
                    nc.scalar.activation(
                        out=sq[:, s0 : s0 + n, :], in_=psmg[:, s0 : s0 + n, :],
                        func=mybir.ActivationFunctionType.Square,
                    )
                nc.vector.tensor_add(
                    h16[:, s0 : s0 + n, :],
                    sq[:, s0 : s0 + n, 0:16], sq[:, s0 : s0 + n, 16:32],
                )
                nc.vector.tensor_add(
                    h8[:, s0 : s0 + n, :],
                    h16[:, s0 : s0 + n, 0:8], h16[:, s0 : s0 + n, 8:16],
                )
                nc.vector.tensor_add(
                    h4[:, s0 : s0 + n, :],
                    h8[:, s0 : s0 + n, 0:4], h8[:, s0 : s0 + n, 4:8],
                )
                nc.vector.tensor_add(
                    h2t[:, s0 : s0 + n, :],
                    h4[:, s0 : s0 + n, 0:2], h4[:, s0 : s0 + n, 2:4],
                )
            nc.vector.tensor_add(
                d2all[:, a0 : a0 + an], h2t[:, :an, 0], h2t[:, :an, 1]
            )
            if bn > 0:
                nc.vector.tensor_add(
                    d2all[:, b0 : b0 + bn],
                    h2t[:, JMG : JMG + bn, 0],
                    h2t[:, JMG : JMG + bn, 1],
                )
            if oc in (nout // 2 - 1, (3 * nout) // 4 - 1):
                # overlap most of the per-point finals with pass 2
                emit_finals_segment(seg_done[0], a0 + an, seg_done[1], b0 + bn)
                seg_done = (a0 + an, b0 + bn)

        # ---------- inter + reg (tiny, replicated) ----------
        mup = wpool.tile([K, D], F32, tag="mup")
        nc.vector.tensor_scalar_add(mup, mu, EPS)
        qsc = wpool.tile([K, D], F32, tag="qsc")
        nc.vector.tensor_mul(qsc, mu, mu)
        q = wpool.tile([K, 1], F32, tag="q")
        nc.vector.tensor_reduce(
            out=q, in_=qsc, axis=mybir.AxisListType.X, op=mybir.AluOpType.add
        )
        qpsc = wpool.tile([K, D], F32, tag="qpsc")
        nc.vector.tensor_mul(qpsc, mup, mup)
        qp = wpool.tile([K, 1], F32, tag="qp")
        nc.vector.tensor_reduce(
            out=qp, in_=qpsc, axis=mybir.AxisListType.X, op=mybir.AluOpType.add
        )
        # pd2[a,b] = qp_a - 2*mup_a.mu_b + q_b via one [64,64] matmul
        ab = wpool.tile([K, D + 2], F32, tag="ab")  # [-2*mup | qp | 1]
        nc.gpsimd.tensor_scalar_mul(ab[:, :D], mup, -2.0)
        nc.gpsimd.tensor_scalar_add(ab[:, D : D + 1], qp, 0.0)
        nc.vector.memset(ab[:, D + 1 : D + 2], 1.0)
        bb = wpool.tile([K, D + 2], F32, tag="bb")  # [mu | 1 | q]
        nc.gpsimd.tensor_scalar_add(bb[:, :D], mu, 0.0)
        nc.vector.memset(bb[:, D : D + 1], 1.0)
        nc.gpsimd.tensor_scalar_add(bb[:, D + 1 : D + 2], q, 0.0)
        psT = psS.tile([D + 2, K], F32, tag="small")
        nc.tensor.transpose(psT, ab, id64)
        atp = wpool.tile([D + 2, K], F32, tag="atp")
        nc.gpsimd.tensor_scalar_add(atp, psT, 0.0)
        psT2 = psS.tile([D + 2, K], F32, tag="small")
        nc.tensor.transpose(psT2, bb, id64)
        btp = wpool.tile([D + 2, K], F32, tag="btp")
        nc.gpsimd.tensor_scalar_add(btp, psT2, 0.0)
        psPD = psS.tile([K, K], F32, tag="small")
        nc.tensor.matmul(psPD, atp, btp)
        pdc = wpool.tile([K, K], F32, tag="pdc")
        nc.vector.tensor_scalar_max(pdc, psPD, 0.0)
        pdist = wpool.tile([K, K], F32, tag="pdist")
        nc.scalar.activation(
            out=pdist, in_=pdc, func=mybir.ActivationFunctionType.Sqrt
        )
        hingeI = wpool.tile([K, K], F32, tag="hingeI")
        nc.scalar.activation(
            out=hingeI, in_=pdist, func=mybir.ActivationFunctionType.Relu,
            bias=float(INTER_MARGIN2), scale=-1.0,
        )
        hm = wpool.tile([K, K], F32, tag="hm")
        nc.vector.tensor_mul(hm, hingeI, eyeneg)
        hm2 = wpool.tile([K, K], F32, tag="hm2")
        nc.vector.tensor_mul(hm2, hm, hm)
        interp = wpool.tile([K, 1], F32, tag="interp")
        nc.vector.tensor_reduce(
            out=interp, in_=hm2, axis=mybir.AxisListType.X, op=mybir.AluOpType.add
        )
        sqp = wpool.tile([K, 1], F32, tag="sqp")
        nc.scalar.activation(
            out=sqp, in_=qp, func=mybir.ActivationFunctionType.Sqrt
        )
        cat2 = wpool.tile([K, 2], F32, tag="cat2")
        nc.gpsimd.tensor_scalar_add(cat2[:, 0:1], interp, 0.0)
        nc.gpsimd.tensor_scalar_add(cat2[:, 1:2], sqp, 0.0)
        psIR = psS.tile([1, 2], F32, tag="small")
        nc.tensor.matmul(psIR, ones64, cat2)
        ir = wpool.tile([1, 2], F32, tag="ir")  # [inter_sum, reg_sum]
        nc.gpsimd.tensor_scalar_add(ir, psIR, 0.0)

        # ---------- intra finals (last segment + accumulation) ----------
        emit_finals_segment(seg_done[0], na, seg_done[1], tpc)
        intra = wpool.tile([1, 1], F32, tag="intra")
        nc.gpsimd.tensor_reduce(
            out=intra, in_=racc_tiles[-1], axis=mybir.AxisListType.C,
            op=mybir.AluOpType.add,
        )
        tot = wpool.tile([1, 3], F32, tag="tot")
        nc.scalar.copy(out=tot[:, 0:1], in_=intra)
        nc.scalar.copy(out=tot[:, 1:3], in_=ir)
        nc.sync.dma_start(out=out_d, in_=tot[0:1, :])

    nc.compile()
    return nc


_NC_CACHE = {}


def _get_program(tpc):
    if tpc not in _NC_CACHE:
        _NC_CACHE[tpc] = build_program(tpc)
    return _NC_CACHE[tpc]


def kernel(features, labels, num_clusters):
    features = np.asarray(features)
    labels = np.asarray(labels)
    n_total = features.shape[0]
    n_core = n_total // N_CORES
    tpc = math.ceil(n_core / P)
    nc = _get_program(tpc)
    in_maps = _host_prep(features, labels, tpc)
    res = run_bass_kernel_spmd(nc, in_maps, list(range(N_CORES)))
    intra_sum = sum(float(res.results[c]["out"][0]) for c in range(N_CORES))
    inter_sum = float(res.results[0]["out"][1])
    reg_sum = float(res.results[0]["out"][2])
    total = (
        intra_sum / K
        + inter_sum / (K * (K - 1))
        + 0.001 * reg_sum / K
    )
    return np.float32(total)


# revision 21
# speedup vs baseline: 1.6992x; 1.0080x over previous
# kernel.py — DiscriminativeLoss on 8 TRN2 NeuronCores (Bass/Tile, SPMD).
#
# Math (matches reference):
#   counts_k = #{i: l_i = k};  S_k = sum_{i in k} x_i;  mu_k = S_k / max(c_k, 1)
#   intra = (1/K) * sum_i invc_{l_i} * relu(||x_i - mu_{l_i} + eps|| - 1.5)^2
#   inter = sum_{a != b} relu(1 - ||(mu_a + eps) - mu_b||)^2 / (K*(K-1))
#   reg   = (1/K) * sum_k ||mu_k + eps||
#   total = intra + inter + 0.001 * reg
#
# Device strategy (per core, data-parallel over points; point i = p*tpc + j
# lives at [p, j]):
#   pass 1: one-hot H2 [128, 64, J1] built per chunk via a single DVE
#     tensor_tensor is_equal against a materialized replicated iota (all
#     operands packed 2-byte -> 2x DVE mode); per-tile PE matmul
#     lhsT=H2[:, :, j] [128, 64] x rhs=xe[:, j, :] [128, 33] accumulates
#     S^T|counts [64, 33] directly (N=33 -> cheap).
#   AllReduce [64, 33] across 8 cores (28us fixed cost; overlapped with
#     pass-2 one-hot prebuilds).
#   stats: invc = 1/max(c,1), mu = S*invc, table [128, 33] = [mu-eps | invc]
#     replicated to rows 64:128 for the B-half pairing.
#   pass 2, per outer chunk of 15 A-tiles + 15 B-tiles: transposed one-hot
#     ht [128, 15*128] built at 4x DVE (TensorScalarPtr is_equal vs the
#     per-partition iota) from a broadcast-DMA'd label row; 3 matmuls per
#     tile accumulate  psum[:, slot, 0:32] = gather(mu-eps) - x  (the diff
#     computed entirely on PE via a -Identity matmul) and
#     psum[:, slot, 32] = gather(invc); Act squares the PSUM diff to bf16;
#     DVE reduces via log2 halving adds (packed bf16 -> 2x mode).
#   finals: dist = sqrt(d2); hinge = relu(dist-1.5); intra partial =
#     sum hinge^2 * invc via 2 muls + row reduce + partition reduce.
#   inter/reg (KxK) replicated on every core from the reduced stats.
import math
import numpy as np
from contextlib import ExitStack

import concourse.bass as bass
import concourse.bacc as bacc
import concourse.tile as tile
import concourse.mybir as mybir
from concourse.bass_utils import run_bass_kernel_spmd

F32 = mybir.dt.float32
BF16 = mybir.dt.bfloat16
I16 = mybir.dt.int16

N_CORES = 8
K = 64
D = 32
P = 128
EPS = 1e-8
PAD_LABEL = 999  # never matches any one-hot row (0..127)

INTRA_MARGIN = 1.5
INTER_MARGIN2 = 1.0  # 2 * 0.5

J1 = 30       # pass-1 tiles per one-hot chunk
JMG = 15      # pass-2 A-tiles (and B-tiles) per outer chunk
PREBUILD = 21   # pass-2 ht chunks emitted before the collective section
L2_BUFS = 6
HT_BUFS = 21


def _host_prep(features, labels, tpc):
    """Shard + relayout on host. Returns per-core input dicts."""
    n_total = features.shape[0]
    n_core = n_total // N_CORES
    n_pad = P * tpc
    import ml_dtypes

    na = (tpc + 1) // 2
    nout = math.ceil(na / JMG)
    iota_rep = np.tile(
        np.arange(K, dtype=np.int16)[None, :, None], (P, 1, J1)
    )
    iotacol = np.arange(P, dtype=np.float32).reshape(P, 1)
    negid = (-np.eye(P)).astype(ml_dtypes.bfloat16)
    id64 = np.eye(K, dtype=np.float32)
    eyeneg = (1.0 - np.eye(K, dtype=np.float32)).astype(ml_dtypes.bfloat16)

    in_maps = []
    for c in range(N_CORES):
        f = np.asarray(features[c * n_core : (c + 1) * n_core], dtype=np.float32)
        l = np.asarray(labels[c * n_core : (c + 1) * n_core], dtype=np.int64)
        if n_pad > n_core:
            f = np.concatenate([f, np.zeros((n_pad - n_core, D), np.float32)], axis=0)
            l = np.concatenate([l, np.full((n_pad - n_core,), PAD_LABEL, np.int64)])
        # xe: [P, tpc, 32] bf16 (counts come from separate ones-rhs matmuls)
        xe = f.reshape(P, tpc, D).astype(ml_dtypes.bfloat16)
        # p-major labels (pass-1 one-hot): [P, tpc] int16
        lpm = l.reshape(P, tpc).astype(np.int16)
        # tile-major labels for pass 2: ltm [nout, 2, JMG*P] int16,
        # [oc, 0] = A-tile labels, [oc, 1] = B-tile labels + 64.
        ltm_full = l.reshape(P, tpc).T.astype(np.int16)  # [tpc, P]
        ltm = np.full((nout, 2, JMG * P), PAD_LABEL, np.int16)
        for oc in range(nout):
            a0 = oc * JMG
            an = min(JMG, na - a0)
            ltm[oc, 0, : an * P] = ltm_full[a0 : a0 + an].ravel()
            b0 = na + a0
            bn = max(0, min(JMG, tpc - b0))
            if bn > 0:
                ltm[oc, 1, : bn * P] = ltm_full[b0 : b0 + bn].ravel() + 64
        in_maps.append(
            {
                "xe": np.ascontiguousarray(xe),
                "lpm": np.ascontiguousarray(lpm),
                "ltm": np.ascontiguousarray(ltm),
                "iota_rep": iota_rep,
                "iotacol": iotacol,
                "negid": negid,
                "id64": id64,
                "eyeneg": eyeneg,
            }
        )
    return in_maps


def build_program(tpc):
    """Build the SPMD Bass program. tpc = tiles per core (cols per partition)."""
    nc = bacc.Bacc(
        "TRN2", target_bir_lowering=False, debug=False, num_devices=N_CORES
    )
    core_ids = list(range(N_CORES))

    na = (tpc + 1) // 2
    nout = math.ceil(na / JMG)
    n_chunks1 = math.ceil(tpc / J1)

    xe_d = nc.dram_tensor("xe", [P, tpc, D], BF16, kind="ExternalInput").ap()
    lpm_d = nc.dram_tensor("lpm", [P, tpc], I16, kind="ExternalInput").ap()
    ltm_d = nc.dram_tensor("ltm", [nout, 2, JMG * P], I16, kind="ExternalInput").ap()
    iota_rep_d = nc.dram_tensor("iota_rep", [P, K, J1], I16, kind="ExternalInput").ap()
    iotacol_d = nc.dram_tensor("iotacol", [P, 1], F32, kind="ExternalInput").ap()
    negid_d = nc.dram_tensor("negid", [P, P], BF16, kind="ExternalInput").ap()
    id64_d = nc.dram_tensor("id64", [K, K], F32, kind="ExternalInput").ap()
    eyeneg_d = nc.dram_tensor("eyeneg", [K, K], BF16, kind="ExternalInput").ap()
    out_d = nc.dram_tensor("out", [3], F32, kind="ExternalOutput").ap()

    with tile.TileContext(nc, num_cores=N_CORES) as tc, ExitStack() as ctx:
        singles = ctx.enter_context(tc.tile_pool(name="singles", bufs=1))
        xpool = ctx.enter_context(tc.tile_pool(name="xpool", bufs=1))
        hpool = ctx.enter_context(tc.tile_pool(name="hpool", bufs=3))
        l2pool = ctx.enter_context(tc.tile_pool(name="l2pool", bufs=L2_BUFS))
        htpool = ctx.enter_context(tc.tile_pool(name="htpool", bufs=HT_BUFS))
        sqpool = ctx.enter_context(tc.tile_pool(name="sqpool", bufs=3))
        hvpool = ctx.enter_context(tc.tile_pool(name="hvpool", bufs=3))
        wpool = ctx.enter_context(tc.tile_pool(name="wpool", bufs=2))
        psA = ctx.enter_context(tc.tile_pool(name="psA", bufs=1, space="PSUM"))
        psMg = ctx.enter_context(tc.tile_pool(name="psMg", bufs=3, space="PSUM"))
        psS = ctx.enter_context(tc.tile_pool(name="psS", bufs=1, space="PSUM"))
        dram = ctx.enter_context(tc.tile_pool(name="dram", bufs=2, space="DRAM"))

        # ---------- constants (critical first: pass-1 inputs) ----------
        lpm = singles.tile([P, tpc], I16)
        nc.sync.dma_start(out=lpm, in_=lpm_d)
        iota_rep = singles.tile([P, K, J1], I16)
        nc.sync.dma_start(out=iota_rep, in_=iota_rep_d)
        margneg = singles.tile([P, 1], F32)
        nc.vector.memset(margneg, -float(INTRA_MARGIN))
        ones64 = singles.tile([K, 1], F32)
        nc.vector.memset(ones64, 1.0)
        # prewarm the Act function table with Sqrt: narrows the possible
        # table sets to one containing sqrt+square+relu+copy, so the single
        # 1.3us load happens here, off-critical, and never again
        actwarm = singles.tile([1, 1], F32)
        nc.scalar.activation(
            out=actwarm, in_=ones64[0:1, :],
            func=mybir.ActivationFunctionType.Sqrt,
        )
        xe = xpool.tile([P, tpc, D], BF16)
        ones128 = singles.tile([P, 1], BF16)
        nc.vector.memset(ones128, 1.0)

        d2all = singles.tile([P, tpc], BF16)
        invc_all = singles.tile([P, tpc], BF16)

        # ---------- pass 1: segment sums S^T [64, 32] + counts [64, 1] ----------
        psumS = psA.tile([K, D], F32)
        psumC = psS.tile([K, 1], F32, tag="small")
        t_done = 0
        for c in range(n_chunks1):
            j0 = c * J1
            jn = min(J1, tpc - j0)
            nc.sync.dma_start(
                out=xe[:, j0 : j0 + jn, :], in_=xe_d[:, j0 : j0 + jn, :]
            )
            h2 = hpool.tile([P, K, J1], BF16, tag="h2")
            eng = nc.gpsimd if (c % 3 == 2) else nc.vector
            eng.tensor_tensor(
                h2[:, :, :jn],
                lpm[:, None, j0 : j0 + jn].to_broadcast((P, K, jn)),
                iota_rep[:, :, :jn],
                mybir.AluOpType.is_equal,
            )
            for j in range(jn):
                nc.tensor.matmul(
                    psumS,
                    h2[:, :, j],
                    xe[:, j0 + j, :],
                    start=(t_done == 0),
                    stop=(t_done == tpc - 1),
                )
                nc.tensor.matmul(
                    psumC,
                    h2[:, :, j],
                    ones128,
                    start=(t_done == 0),
                    stop=(t_done == tpc - 1),
                )
                t_done += 1

        # ---------- remaining constants (needed only from pass 2 on) ----------
        iotacol = singles.tile([P, 1], F32)
        nc.sync.dma_start(out=iotacol, in_=iotacol_d)
        negid = singles.tile([P, P], BF16)
        nc.sync.dma_start(out=negid, in_=negid_d)
        id64 = singles.tile([K, K], F32)
        nc.sync.dma_start(out=id64, in_=id64_d)
        eyeneg = singles.tile([K, K], BF16)
        nc.sync.dma_start(out=eyeneg, in_=eyeneg_d)

        # ---------- pass-2 prep: prebuild label rows + transposed one-hots ----
        # (no dependency on the collective -> fills the AllReduce window)
        l2_tiles = {}
        ht_tiles = {}

        def emit_l2_ht(oc, eng=nc.vector):
            src = ltm_d[oc]
            l2 = l2pool.tile([P, JMG * P], I16, tag="l2")
            nc.sync.dma_start(
                out=l2,
                in_=bass.AP(
                    tensor=src.tensor,
                    offset=src.offset,
                    ap=[[JMG * P, 2], [0, K]] + [[1, JMG * P]],
                ),
            )
            ht = htpool.tile([P, JMG * P], BF16, tag="ht")
            eng.tensor_single_scalar(
                ht, l2, iotacol, mybir.AluOpType.is_equal
            )
            l2_tiles[oc] = l2
            ht_tiles[oc] = ht

        # ---------- AllReduce the [64, 33] stats ----------
        sg_local = wpool.tile([K, D + 1], F32, tag="sg")
        nc.gpsimd.tensor_scalar_add(sg_local[:, :D], psumS, 0.0)
        nc.gpsimd.tensor_scalar_add(sg_local[:, D : D + 1], psumC, 0.0)
        cc_in = dram.tile([K, D + 1], F32)
        cc_out = dram.tile([K, D + 1], F32)
        nc.gpsimd.dma_start(out=cc_in, in_=sg_local)
        nc.gpsimd.collective_compute(
            "AllReduce",
            mybir.AluOpType.add,
            replica_groups=[core_ids],
            ins=[cc_in.opt()],
            outs=[cc_out.opt()],
        )
        sg = singles.tile([K, D + 1], F32)
        nc.gpsimd.dma_start(out=sg, in_=cc_out)

        # ---------- critical stats: table = [mu - eps | invc] ----------
        safec = singles.tile([K, 1], F32)
        nc.vector.tensor_scalar_max(safec, sg[:, D : D + 1], 1.0)
        invc = singles.tile([K, 1], F32)
        nc.vector.reciprocal(invc, safec)
        mu = singles.tile([K, D], F32)
        nc.vector.tensor_mul(mu, sg[:, :D], invc.to_broadcast((K, D)))
        table = singles.tile([P, D + 1], BF16)
        nc.vector.tensor_scalar_add(table[:K, :D], mu, -EPS)
        nc.vector.tensor_scalar_add(table[:K, D : D + 1], invc, 0.0)
        # replicate rows [0,64) -> [64,128) for the B-half matmuls
        # (issued from the Pool queue, which is idle post-collective; keeping
        # it off SP avoids stalling the pass-2 label-row DMA stream)
        nc.gpsimd.dma_start(out=table[K:, :], in_=table[:K, :])

        # ---------- PE keep-warm during the collective ----------
        # the tensor engine p-state drops after ~idle; feed it junk matmuls
        # (into the recycled small-psum slot) so pass-2 gathers start at
        # full clock. No data deps; they fill the AllReduce window.
        junkps = psS.tile([K, 512], F32, tag="small")
        for w in range(120):
            nc.tensor.matmul(
                junkps, negid[:, :K], xe[:, 0:16, :], start=True, stop=True
            )

        # ---------- pass-2 one-hot prebuilds (fill the AllReduce window) ----
        for oc in range(min(PREBUILD, nout)):
            emit_l2_ht(oc)

        # per-point finals tiles + helper (emitted in segments so most of
        # the sqrt/relu/mul/accumulate work overlaps pass 2). The running
        # per-partition intra sum is chained through tensor_tensor_reduce's
        # accumulator seed.
        dist = singles.tile([P, tpc], BF16)
        hinge = singles.tile([P, tpc], BF16)
        hsq = dist  # dist is dead after the relu; reuse its storage
        hv = singles.tile([P, tpc], BF16)
        seg_done = (0, na)
        racc_tiles = []

        def emit_finals_segment(ca0, ca1, cb0, cb1):
            for c0, c1 in ((ca0, ca1), (cb0, cb1)):
                if c1 <= c0:
                    continue
                nc.scalar.activation(
                    out=dist[:, c0:c1], in_=d2all[:, c0:c1],
                    func=mybir.ActivationFunctionType.Sqrt,
                )
                nc.scalar.activation(
                    out=hinge[:, c0:c1], in_=dist[:, c0:c1],
                    func=mybir.ActivationFunctionType.Relu, bias=margneg,
                )
                nc.vector.tensor_mul(
                    hsq[:, c0:c1], hinge[:, c0:c1], hinge[:, c0:c1]
                )
                racc = singles.tile([P, 1], F32, tag=f"racc{len(racc_tiles)}")
                nc.vector.tensor_tensor_reduce(
                    out=hv[:, c0:c1], in0=hsq[:, c0:c1],
                    in1=invc_all[:, c0:c1], scale=1.0,
                    scalar=(racc_tiles[-1] if racc_tiles else 0.0),
                    op0=mybir.AluOpType.mult, op1=mybir.AluOpType.add,
                    accum_out=racc,
                )
                racc_tiles.append(racc)

        # ---------- pass 2: diff via PE, square on Act, reduce on DVE ----------
        for oc in range(nout):
            a0 = oc * JMG
            an = min(JMG, na - a0)
            b0 = na + a0
            bn = max(0, min(JMG, tpc - b0))
            if oc not in ht_tiles:
                # in-loop one-hot builds go to the (otherwise idle) Pool
                # engine; DVE keeps the halving reduce
                emit_l2_ht(oc, nc.gpsimd)
            ht = ht_tiles.pop(oc)
            l2_tiles.pop(oc)

            # psmg [P, 32, 32] = exactly 2 psum banks; 32-wide diff slots never
            # cross a bank boundary. Slots 0:15 = A diffs, 15:30 = B diffs,
            # slot 30 col i = A-tile-i invc, slot 31 col i = B-tile-i invc.
            psmg = psMg.tile([P, 2 * JMG + 2, D], F32, tag="psmg")
            for i in range(an):
                hta = ht[:K, i * P : (i + 1) * P]
                # gather(mu-eps) opens the group; negid accumulates -x
                nc.tensor.matmul(
                    psmg[:, i, :], hta, table[:K, :D], start=True, stop=False
                )
                nc.tensor.matmul(
                    psmg[:, i, :], negid, xe[:, a0 + i, :],
                    start=False, stop=True,
                )
                nc.tensor.matmul(
                    psmg[:, 2 * JMG, i : i + 1], hta, table[:K, D : D + 1],
                    start=True, stop=True,
                )
            for i in range(bn):
                htb = ht[K:, i * P : (i + 1) * P]
                s = JMG + i
                nc.tensor.matmul(
                    psmg[:, s, :], htb, table[K:, :D], start=True, stop=False
                )
                nc.tensor.matmul(
                    psmg[:, s, :], negid, xe[:, b0 + i, :],
                    start=False, stop=True,
                )
                nc.tensor.matmul(
                    psmg[:, 2 * JMG + 1, i : i + 1], htb, table[K:, D : D + 1],
                    start=True, stop=True,
                )

            # invc gather extraction (Pool engine)
            nc.gpsimd.tensor_scalar_add(
                invc_all[:, a0 : a0 + an], psmg[:, 2 * JMG, :an], 0.0
            )
            if bn > 0:
                nc.gpsimd.tensor_scalar_add(
                    invc_all[:, b0 : b0 + bn], psmg[:, 2 * JMG + 1, :bn], 0.0
                )
            # square + halving-add reduction over D=32 (packed bf16 -> 2x DVE)
            sq = sqpool.tile([P, 2 * JMG, D], BF16, tag="sq")
            h16 = hvpool.tile([P, 2 * JMG, 16], BF16, tag="h16")
            h8 = hvpool.tile([P, 2 * JMG, 8], BF16, tag="h8")
            h4 = hvpool.tile([P, 2 * JMG, 4], BF16, tag="h4")
            h2t = hvpool.tile([P, 2 * JMG, 2], BF16, tag="h2t")
            if an == JMG and bn == JMG:
                ranges = [(0, 2 * JMG)]  # one combined pass over all 30 slots
            else:
                ranges = [(0, an)] + ([(JMG, bn)] if bn > 0 else [])
            for s0, n in ranges:
                if oc % 16 == 7:
                    # a few squares on DVE to keep Act off the critical path
                    nc.vector.tensor_mul(
                        sq[:, s0 : s0 + n, :],
                        psmg[:, s0 : s0 + n, :], psmg[:, s0 : s0 + n, :],
                    )
                else:
                    nc.scalar.activation(
                        out=sq[:, s0 : s0 + n, :], in_=psmg[:, s0 : s0 + n, :],
                        func=mybir.ActivationFunctionType.Square,
                    )
                nc.vector.tensor_add(
                    h16[:, s0 : s0 + n, :],
                    sq[:, s0 : s0 + n, 0:16], sq[:, s0 : s0 + n, 16:32],
                )
                nc.vector.tensor_add(
                    h8[:, s0 : s0 + n, :],
                    h16[:, s0 : s0 + n, 0:8], h16[:, s0 : s0 + n, 8:16],
                )
                nc.vector.tensor_add(
                    h4[:, s0 : s0 + n, :],
                    h8[:, s0 : s0 + n, 0:4], h8[:, s0 : s0 + n, 4:8],
                )
                nc.vector.tensor_add(
                    h2t[:, s0 : s0 + n, :],
                    h4[:, s0 : s0 + n, 0:2], h4[:, s0 : s0 + n, 2:4],
                )
            nc.vector.tensor_add(
                d2all[:, a0 : a0 + an], h2t[:, :an, 0], h2t[:, :an, 1]
            )
            if bn > 0:
                nc.vector.tensor_add(
                    d2all[:, b0 : b0 + bn],
                    h2t[:, JMG : JMG + bn, 0],
                    h2t[:, JMG : JMG + bn, 1],
                )
            if oc in (nout // 2 - 1, (3 * nout) // 4 - 1):
                # overlap most of the per-point finals with pass 2
                emit_finals_segment(seg_done[0], a0 + an, seg_done[1], b0 + bn)
                seg_done = (a0 + an, b0 + bn)

        # ---------- inter + reg (tiny, replicated) ----------
        mup = wpool.tile([K, D], F32, tag="mup")
        nc.vector.tensor_scalar_add(mup, mu, EPS)
        qsc = wpool.tile([K, D], F32, tag="qsc")
        nc.vector.tensor_mul(qsc, mu, mu)
        q = wpool.tile([K, 1], F32, tag="q")
        nc.vector.tensor_reduce(
            out=q, in_=qsc, axis=mybir.AxisListType.X, op=mybir.AluOpType.add
        )
        qpsc = wpool.tile([K, D], F32, tag="qpsc")
        nc.vector.tensor_mul(qpsc, mup, mup)
        qp = wpool.tile([K, 1], F32, tag="qp")
        nc.vector.tensor_reduce(
            out=qp, in_=qpsc, axis=mybir.AxisListType.X, op=mybir.AluOpType.add
        )
        # pd2[a,b] = qp_a - 2*mup_a.mu_b + q_b via one [64,64] matmul
        ab = wpool.tile([K, D + 2], F32, tag="ab")  # [-2*mup | qp | 1]
        nc.gpsimd.tensor_scalar_mul(ab[:, :D], mup, -2.0)
        nc.gpsimd.tensor_scalar_add(ab[:, D : D + 1], qp, 0.0)
        nc.vector.memset(ab[:, D + 1 : D + 2], 1.0)
        bb = wpool.tile([K, D + 2], F32, tag="bb")  # [mu | 1 | q]
        nc.gpsimd.tensor_scalar_add(bb[:, :D], mu, 0.0)
        nc.vector.memset(bb[:, D : D + 1], 1.0)
        nc.gpsimd.tensor_scalar_add(bb[:, D + 1 : D + 2], q, 0.0)
        psT = psS.tile([D + 2, K], F32, tag="small")
        nc.tensor.transpose(psT, ab, id64)
        atp = wpool.tile([D + 2, K], F32, tag="atp")
        nc.gpsimd.tensor_scalar_add(atp, psT, 0.0)
        psT2 = psS.tile([D + 2, K], F32, tag="small")
        nc.tensor.transpose(psT2, bb, id64)
        btp = wpool.tile([D + 2, K], F32, tag="btp")
        nc.gpsimd.tensor_scalar_add(btp, psT2, 0.0)
        psPD = psS.tile([K, K], F32, tag="small")
        nc.tensor.matmul(psPD, atp, btp)
        pdc = wpool.tile([K, K], F32, tag="pdc")
        nc.vector.tensor_scalar_max(pdc, psPD, 0.0)
        pdist = wpool.tile([K, K], F32, tag="pdist")
        nc.scalar.activation(
            out=pdist, in_=pdc, func=mybir.ActivationFunctionType.Sqrt
        )
        hingeI = wpool.tile([K, K], F32, tag="hingeI")
        nc.scalar.activation(
            out=hingeI, in_=pdist, func=mybir.ActivationFunctionType.Relu,
            bias=float(INTER_MARGIN2), scale=-1.0,
        )
        hm = wpool.tile([K, K], F32, tag="hm")
        nc.vector.tensor_mul(hm, hingeI, eyeneg)
        hm2 = wpool.tile([K, K], F32, tag="hm2")
        nc.vector.tensor_mul(hm2, hm, hm)
        interp = wpool.tile([K, 1], F32, tag="interp")
        nc.vector.tensor_reduce(
            out=interp, in_=hm2, axis=mybir.AxisListType.X, op=mybir.AluOpType.add
        )
        sqp = wpool.tile([K, 1], F32, tag="sqp")
        nc.scalar.activation(
            out=sqp, in_=qp, func=mybir.ActivationFunctionType.Sqrt
        )
        cat2 = wpool.tile([K, 2], F32, tag="cat2")
        nc.gpsimd.tensor_scalar_add(cat2[:, 0:1], interp, 0.0)
        nc.gpsimd.tensor_scalar_add(cat2[:, 1:2], sqp, 0.0)
        psIR = psS.tile([1, 2], F32, tag="small")
        nc.tensor.matmul(psIR, ones64, cat2)
        ir = wpool.tile([1, 2], F32, tag="ir")  # [inter_sum, reg_sum]
        nc.gpsimd.tensor_scalar_add(ir, psIR, 0.0)

        # ---------- intra finals (last segment + accumulation) ----------
        emit_finals_segment(seg_done[0], na, seg_done[1], tpc)
        intra = wpool.tile([1, 1], F32, tag="intra")
        nc.gpsimd.tensor_reduce(
            out=intra, in_=racc_tiles[-1], axis=mybir.AxisListType.C,
            op=mybir.AluOpType.add,
        )
        tot = wpool.tile([1, 3], F32, tag="tot")
        nc.scalar.copy(out=tot[:, 0:1], in_=intra)
        nc.scalar.copy(out=tot[:, 1:3], in_=ir)
        nc.sync.dma_start(out=out_d, in_=tot[0:1, :])

    nc.compile()
    return nc


_NC_CACHE = {}


def _get_program(tpc):
    if tpc not in _NC_CACHE:
        _NC_CACHE[tpc] = build_program(tpc)
    return _NC_CACHE[tpc]


def kernel(features, labels, num_clusters):
    features = np.asarray(features)
    labels = np.asarray(labels)
    n_total = features.shape[0]
    n_core = n_total // N_CORES
    tpc = math.ceil(n_core / P)
    nc = _get_program(tpc)
    in_maps = _host_prep(features, labels, tpc)
    res = run_bass_kernel_spmd(nc, in_maps, list(range(N_CORES)))
    intra_sum = sum(float(res.results[c]["out"][0]) for c in range(N_CORES))
    inter_sum = float(res.results[0]["out"][1])
    reg_sum = float(res.results[0]["out"][2])
    total = (
        intra_sum / K
        + inter_sum / (K * (K - 1))
        + 0.001 * reg_sum / K
    )
    return np.float32(total)
